# revision 7
# baseline (speedup 1.0000x reference)
"""Trainium2 Bass kernel for a dense transformer block (B=4, T=2048, C=1024, 16 heads).

Sharding over 8 NeuronCores: core i handles batch b=i//2 with shard s=i%2.
 - LN1 + QKV + causal attention for its 8 heads (c-slice [512s, 512s+512)) over full T
 - exchange of attention outputs within the (b) pair via 4 halved
   ReduceScatter ops (zero-region trick, fully SPMD-symmetric)
 - proj + LN2 + FFN + residuals on its t-half rows [1024s, 1024s+1024)

All GEMMs run in bf16 with fp32 PSUM accumulation. LayerNorm gain/bias are
folded into the weight matrices on the host; LN statistics use bn_stats and
rsqrt is exp(-0.5*ln(var+eps)) so all ACT functions share one table set.

Scheduling notes (engine queues are strict FIFO, so emission order matters):
 - PSUM uses two shared tags: "A" = 3x [128,1024] slots (GEMM-chain
   accumulators, QK scores) and "B" = 2x [128,512] slots (PE transposes, PV
   accumulators). Chains write both 512-wide halves of an "A" slot and are
   evacuated by a single wide ACT/DVE op.
 - q/k GEMM chains for head-pair hp+1 are emitted inside head-pair hp's
   attention loop, filling PE bubbles while the scalar engine runs exp().
 - The attention kk loop is software-pipelined (PV one step behind QK).
 - proj leaves the RS3 head-pair contribution to a second pass interleaved
   with LN2, so the PE never waits on the last ReduceScatter.
"""

from contextlib import ExitStack

import ml_dtypes
import numpy as np

import concourse.bass as bass
import concourse.mybir as mybir
import concourse.tile as tile
from concourse import bacc
from concourse.bass_utils import run_bass_kernel_spmd

f32 = mybir.dt.float32
bf16 = mybir.dt.bfloat16
AF = mybir.ActivationFunctionType
ALU = mybir.AluOpType
AX = mybir.AxisListType

B, T, C = 4, 2048, 1024
NH, D = 16, 64
F = 4 * C
H = C // 2            # per-core head c-slice (8 heads)
TH = T // 2           # per-core t-half for proj/FFN
EPS = 1e-5
RG = [[0, 1], [2, 3], [4, 5], [6, 7]]

_CACHE = {}


class S:
    """Shared build state."""
    pass


def _layernorm_tile(nc, st, xt, dst):
    """Row-standardize xt [128, C] -> dst [128, C] (bn_stats + ln/exp rsqrt)."""
    work = st.work
    st6 = work.tile([128, 2, 6], f32, name="st6", tag="st6")
    xg = xt[:].rearrange("p (g n) -> p g n", g=2)
    nc.vector.bn_stats(st6[:, 0, :], xg[:, 0, :])
    nc.vector.bn_stats(st6[:, 1, :], xg[:, 1, :])
    mv = work.tile([128, 2], f32, name="mv", tag="mv")
    nc.vector.bn_aggr(mv[:], st6[:])
    veps = work.tile([128, 1], f32, name="veps", tag="veps")
    nc.vector.tensor_scalar_add(veps[:], mv[:, 1:2], EPS)
    lv = work.tile([128, 1], f32, name="lv", tag="lv")
    nc.scalar.activation(lv[:], veps[:], AF.Ln)
    rsig = work.tile([128, 1], f32, name="rsig", tag="rsig")
    nc.scalar.activation(rsig[:], lv[:], AF.Exp, scale=-0.5)
    nmu = work.tile([128, 1], f32, name="nmu", tag="nmu")
    nc.vector.tensor_tensor(nmu[:], mv[:, 0:1], rsig[:], ALU.mult)
    nc.vector.tensor_scalar_mul(nmu[:], nmu[:], -1.0)
    nc.scalar.activation(dst[:], xt[:], AF.Identity, bias=nmu[:], scale=rsig[:])


def _phase_hcol(nc, st):
    """LN1 + transpose (hcol = h^T for all 4 chunks) and the v GEMMs."""
    ps_t = st.ps_t
    st.qkvp = st.tc.tile_pool(name="qkv", bufs=1)
    qkv = st.qkvp.__enter__()
    st.wqkvp = st.tc.tile_pool(name="wqkv", bufs=1)
    wqkv = st.wqkvp.__enter__()
    st.htcp = st.tc.tile_pool(name="htc", bufs=1)
    htc = st.htcp.__enter__()
    st.xhp = st.tc.tile_pool(name="xh", bufs=3)
    xh = st.xhp.__enter__()

    st.wq_sb = [wqkv.tile([128, H], bf16, name=f"wq{k}", tag=f"wq{k}")
                for k in range(8)]
    st.wk_sb = [wqkv.tile([128, H], bf16, name=f"wk{k}", tag=f"wk{k}")
                for k in range(8)]
    wv_sb = [wqkv.tile([128, H], bf16, name=f"wv{k}", tag=f"wv{k}")
             for k in range(8)]
    for k in range(8):
        nc.gpsimd.dma_start(st.wq_sb[k][:], st.wq_h[k * 128:(k + 1) * 128, :])
        nc.gpsimd.dma_start(st.wk_sb[k][:], st.wk_h[k * 128:(k + 1) * 128, :])
        nc.gpsimd.dma_start(wv_sb[k][:], st.wv_h[k * 128:(k + 1) * 128, :])

    st.qT = [qkv.tile([128, T], bf16, name=f"qT{i}", tag=f"qT{i}") for i in range(4)]
    st.kT = [qkv.tile([128, T], bf16, name=f"kT{i}", tag=f"kT{i}") for i in range(4)]
    st.vn = [qkv.tile([128, 520], bf16, name=f"vn{i}", tag=f"vn{i}")
             for i in range(16)]
    st.hcol = [htc.tile([128, 8 * 512], bf16, name=f"hcol{j}", tag=f"hcol{j}")
               for j in range(4)]

    for j in range(4):  # t-chunks of 512
        for tt4 in range(4):  # t-tiles of 128 within the chunk
            tt = j * 4 + tt4
            xt = xh.tile([128, C], f32, name="xt", tag="xt")
            nc.sync.dma_start(xt[:], st.x_h[tt * 128:(tt + 1) * 128, :])
            ht = xh.tile([128, C], bf16, name="ht", tag="ht")
            _layernorm_tile(nc, st, xt, ht)
            for cc in range(8):
                ptr = ps_t("B", (128, 128), bf16)
                nc.tensor.transpose(ptr[:], ht[:, cc * 128:(cc + 1) * 128],
                                    st.ident[:])
                nc.vector.tensor_copy(
                    out=st.hcol[j][:, cc * 512 + tt4 * 128:
                                   cc * 512 + (tt4 + 1) * 128],
                    in_=ptr[:])
        # v GEMM for this chunk (natural layout, strided into vn + ones col)
        for tt4 in range(4):
            tt = j * 4 + tt4
            pg = ps_t("A")
            for k in range(8):
                nc.tensor.matmul(
                    pg[:],
                    st.hcol[j][:, k * 512 + tt4 * 128:k * 512 + (tt4 + 1) * 128],
                    wv_sb[k][:], start=(k == 0), stop=False)
            nc.tensor.matmul(pg[:], st.onesr[:, 0:128], st.bv_sb[:],
                             start=False, stop=True)
            nc.scalar.copy(
                st.vn[tt][:, 0:520].rearrange("p (h e) -> p h e", h=8)[:, :, 0:64],
                pg[:].rearrange("p (h d) -> p h d", h=8))
            nc.vector.memset(
                st.vn[tt][:, 0:520].rearrange("p (h e) -> p h e", h=8)[:, :, 64:65],
                1.0)
    st.xhp.__exit__(None, None, None)


def _emit_qk_chain(nc, st, hp, j):
    """q and k GEMMs for (head-pair hp, t-chunk j): one wide PSUM slot."""
    pg = st.ps_t("A", (128, 1024))
    for half, (dstT, wsb, brow) in enumerate(
            ((st.qT, st.wq_sb, st.bq_sb), (st.kT, st.wk_sb, st.bk_sb))):
        sl = pg[:, half * 512:(half + 1) * 512]
        for k in range(8):
            nc.tensor.matmul(sl, wsb[k][:, hp * 128:(hp + 1) * 128],
                             st.hcol[j][:, k * 512:(k + 1) * 512],
                             start=(k == 0), stop=False)
        nc.tensor.matmul(sl, brow[0:1, hp * 128:(hp + 1) * 128],
                         st.onesr[:, 0:512], start=False, stop=True)
        nc.vector.tensor_copy(out=dstT[hp][:, j * 512:(j + 1) * 512], in_=sl)


def _phase_attention(nc, st):
    """Causal attention per head-pair; ships results via ReduceScatter.

    The kk loop is software-pipelined: PV(kk) is emitted after QK(kk+1), and
    the QK score PSUM tag is multi-buffered, so the PE streams QK matmuls
    while the scalar engine exponentiates the previous chunk. q/k GEMMs for
    the NEXT head-pair are interleaved at chunk boundaries.
    """
    ps_t = st.ps_t
    st.wop = st.tc.tile_pool(name="wop", bufs=1, side="right")
    wop = st.wop.__enter__()
    st.attp = st.tc.tile_pool(name="attp", bufs=1, side="right")
    attp = st.attp.__enter__()
    st.attsbp = st.tc.tile_pool(name="attsb", bufs=1, side="right")
    attsb = st.attsbp.__enter__()

    attA = [attp.tile([128, T], bf16, name=f"attA{i}", tag=f"attA{i}")
            for i in range(4)]
    st.attA = attA
    st.att_sb = [attsb.tile([128, TH], bf16, name=f"asb{k}", tag=f"asb{k}")
                 for k in range(4)]
    aw = st.tc.tile_pool(name="aw", bufs=2)
    st.awp = aw
    aw = aw.__enter__()
    st.wo_sb = [wop.tile([128, C], bf16, name=f"wo{k}", tag=f"wo{k}")
                for k in range(8)]
    for k in range(8):
        nc.gpsimd.dma_start(st.wo_sb[k][:], st.wo_h[k * 128:(k + 1) * 128, :])

    def emit_pv(hp, kk, nk, r, ptb, po):
        for bi in range(2):
            h = 2 * hp + bi
            nc.tensor.matmul(
                po[bi][0:65, r:512],
                st.vn[kk][:, 65 * h:65 * h + 65],
                ptb[:, bi * 512 + r:bi * 512 + 512],
                start=(kk == 0), stop=(kk == nk - 1))

    def emit_norm(hp, j, po):
        tq0 = j * 512
        sj = j // 2
        for bi, b0 in enumerate((0, 64)):
            rs_row = aw.tile([1, 512], bf16, name="rs_row", tag="rsrow")
            nc.scalar.copy(rs_row[:], po[bi][64:65, :])
            pb = ps_t("A", (64, 512))
            nc.tensor.matmul(pb[:], st.onesr[:, 0:64], rs_row[:],
                             start=True, stop=True)
            rbi = aw.tile([64, 512], f32, name="rbi", tag="rbi")
            nc.vector.reciprocal_approx_fast(rbi[:], pb[:])
            attF = aw.tile([64, 512], bf16, name="attF", tag="attF")
            nc.vector.tensor_tensor(attF[:], po[bi][0:64, :], rbi[:],
                                    ALU.mult)
            nc.vector.tensor_scalar_mul(
                attA[hp][b0:b0 + 64, tq0:tq0 + 512], attF[:],
                st.sel_sb[0:64, sj:sj + 1])
            attBc = aw.tile([64, 512], bf16, name="attBc", tag="attBc")
            nc.vector.tensor_scalar_mul(attBc[:], attF[:],
                                        st.seln_sb[0:64, sj:sj + 1])
            nc.sync.dma_start(
                st.rs_in[hp][sj, b0:b0 + 64,
                             (j % 2) * 512:(j % 2) * 512 + 512],
                attBc[:])

    for j in range(4):
        _emit_qk_chain(nc, st, 0, j)
    for hp in range(4):
        norm_pending = None
        for j in range(4):
            tq0 = j * 512
            nk = 4 * (j + 1)
            po = [ps_t("pv"), ps_t("pv")]
            pending = None
            for kk in range(nk):
                r = 128 * (kk - 4 * j) if kk >= 4 * j else 0
                pqk = ps_t("A", (128, 1024))
                for bi, b0 in enumerate((0, 64)):
                    nc.tensor.matmul(
                        pqk[:, bi * 512 + r:bi * 512 + 512],
                        st.kT[hp][b0:b0 + 64, kk * 128:(kk + 1) * 128],
                        st.qT[hp][b0:b0 + 64, tq0 + r:tq0 + 512],
                        start=True, stop=True)
                ptb = st.ptp.tile([128, 1024], bf16, name="ptb", tag="pt")
                if r == 0:
                    nc.scalar.activation(ptb[:], pqk[:], AF.Exp)
                else:
                    nc.scalar.activation(
                        ptb[:].rearrange("p (b w) -> p b w", b=2)[:, :, r:512],
                        pqk[:].rearrange("p (b w) -> p b w", b=2)[:, :, r:512],
                        AF.Exp)
                if kk >= 4 * j:
                    nc.vector.tensor_tensor(
                        ptb[:].rearrange("p (b w) -> p b w", b=2)[:, :, r:r + 128],
                        ptb[:].rearrange("p (b w) -> p b w", b=2)[:, :, r:r + 128],
                        st.tri[:, None, :].to_broadcast((128, 2, 128)),
                        ALU.mult)
                if pending is not None:
                    emit_pv(hp, *pending)
                pending = (kk, nk, r, ptb, po)
                if kk == 2 and norm_pending is not None:
                    emit_norm(hp, j - 1, norm_pending)
                    norm_pending = None
            emit_pv(hp, *pending)
            norm_pending = po
            if hp < 3:
                _emit_qk_chain(nc, st, hp + 1, j)
        emit_norm(hp, 3, norm_pending)

        nc.gpsimd.collective_compute(
            "ReduceScatter", ALU.add, replica_groups=RG,
            ins=[st.rs_in[hp][:]], outs=[st.rs_out[hp][:]])
        nc.gpsimd.dma_start(st.att_sb[hp][:], st.rs_out[hp][:])


def _phase_proj(nc, st):
    """Projection pass A: local heads + first three exchanged head-pairs +
    residual. The RS3 contribution is added during LN2 (pass B)."""
    ps_t = st.ps_t
    st.awp.__exit__(None, None, None)
    st.htcp.__exit__(None, None, None)
    st.wqkvp.__exit__(None, None, None)
    st.qkvp.__exit__(None, None, None)
    st.x2p = st.tc.tile_pool(name="x2p", bufs=1)
    x2p = st.x2p.__enter__()
    st.latebp = st.tc.tile_pool(name="lateb", bufs=1)
    lateb = st.latebp.__enter__()
    st.xrpp = st.tc.tile_pool(name="xrp", bufs=2)
    xrp = st.xrpp.__enter__()

    st.b2_sb = lateb.tile([1, C], bf16, name="b2_sb")
    nc.sync.dma_start(st.b2_sb[:], st.b2_h[:])
    st.x2 = [x2p.tile([128, C], f32, name=f"x2_{t}", tag=f"x2_{t}")
             for t in range(8)]
    for tt in range(8):
        xr = xrp.tile([128, C], f32, name="xr", tag="xr")
        nc.sync.dma_start(xr[:], st.xres_h[tt * 128:(tt + 1) * 128, :])
        pg = ps_t("A", (128, 1024))
        for cc in range(2):
            sl = pg[:, cc * 512:(cc + 1) * 512]
            for k in range(4):
                for half in range(2):
                    nc.tensor.matmul(
                        sl,
                        st.attA[k][:, half * TH + tt * 128:
                                   half * TH + (tt + 1) * 128],
                        st.wo_sb[k][:, cc * 512:(cc + 1) * 512],
                        start=(k == 0 and half == 0), stop=False)
            for k in range(3):
                nc.tensor.matmul(sl, st.att_sb[k][:, tt * 128:(tt + 1) * 128],
                                 st.wo_sb[4 + k][:, cc * 512:(cc + 1) * 512],
                                 start=False, stop=(k == 2))
        nc.vector.tensor_tensor(st.x2[tt][:], pg[:], xr[:], ALU.add)
    st.xrpp.__exit__(None, None, None)


def _phase_ln2(nc, st):
    """proj pass B (RS3 head-pair) + LN2 + transpose to h2T, interleaved."""
    ps_t = st.ps_t
    st.h2p = st.tc.tile_pool(name="h2p", bufs=1)
    h2p = st.h2p.__enter__()
    st.h2wp = st.tc.tile_pool(name="h2w", bufs=3)
    h2w = st.h2wp.__enter__()
    st.h2T = [h2p.tile([128, TH], bf16, name=f"h2T{k}", tag=f"h2T{k}")
              for k in range(8)]
    for tt in range(8):
        pgb = ps_t("A", (128, 1024))
        for cc in range(2):
            nc.tensor.matmul(pgb[:, cc * 512:(cc + 1) * 512],
                             st.att_sb[3][:, tt * 128:(tt + 1) * 128],
                             st.wo_sb[7][:, cc * 512:(cc + 1) * 512],
                             start=True, stop=True)
        nc.vector.tensor_tensor(st.x2[tt][:], pgb[:], st.x2[tt][:], ALU.add)
        h2t = h2w.tile([128, C], bf16, name="h2t", tag="h2t")
        _layernorm_tile(nc, st, st.x2[tt], h2t)
        for cc in range(8):
            ptr = ps_t("B", (128, 128), bf16)
            nc.tensor.transpose(ptr[:], h2t[:, cc * 128:(cc + 1) * 128],
                                st.ident[:])
            nc.vector.tensor_copy(out=st.h2T[cc][:, tt * 128:(tt + 1) * 128],
                                  in_=ptr[:])
    st.h2wp.__exit__(None, None, None)
    st.attsbp.__exit__(None, None, None)
    st.attp.__exit__(None, None, None)
    st.wop.__exit__(None, None, None)


def _phase_ffn(nc, st):
    """FFN with grouped ff-dim accumulation, residual, output DMA."""
    ps_t = st.ps_t
    yacp = st.tc.tile_pool(name="yac", bufs=1)
    yac = yacp.__enter__()
    w1pp = st.tc.tile_pool(name="w1p", bufs=16)
    w1p = w1pp.__enter__()
    w2pp = st.tc.tile_pool(name="w2p", bufs=8)
    w2p = w2pp.__enter__()
    utpp = st.tc.tile_pool(name="utp", bufs=8)
    utp = utpp.__enter__()

    y_acc = [yac.tile([128, C], f32, name=f"ya{t}", tag=f"ya{t}")
             for t in range(8)]
    for g in range(4):
        w1g = []
        for k in range(8):
            w1k = w1p.tile([128, 1024], bf16, name="w1k", tag="w1k")
            nc.sync.dma_start(w1k[:],
                              st.w1_h[k * 128:(k + 1) * 128,
                                      g * 1024:(g + 1) * 1024])
            w1g.append(w1k)
        ut_g = []
        for ff in range(8):
            f = g * 8 + ff
            ut = utp.tile([128, TH], bf16, name="ut", tag="ut")
            pg = ps_t("A", (128, 1024))
            for tch in range(2):
                sl = pg[:, tch * 512:(tch + 1) * 512]
                for k in range(8):
                    nc.tensor.matmul(sl, w1g[k][:, ff * 128:(ff + 1) * 128],
                                     st.h2T[k][:, tch * 512:(tch + 1) * 512],
                                     start=(k == 0), stop=(k == 7))
            nc.scalar.activation(ut[:], pg[:], AF.Relu,
                                 bias=st.b1_sb[:, f:f + 1])
            ut_g.append(ut)
        w2g = []
        for ff in range(8):
            f = g * 8 + ff
            w2t = w2p.tile([128, C], bf16, name="w2t", tag="w2t")
            nc.sync.dma_start(w2t[:], st.w2_h[f * 128:(f + 1) * 128, :])
            w2g.append(w2t)
        for tt in range(8):
            pg = ps_t("A", (128, 1024))
            for cc in range(2):
                sl = pg[:, cc * 512:(cc + 1) * 512]
                for ff in range(8):
                    nc.tensor.matmul(sl, ut_g[ff][:, tt * 128:(tt + 1) * 128],
                                     w2g[ff][:, cc * 512:(cc + 1) * 512],
                                     start=(ff == 0),
                                     stop=(False if g == 0 else ff == 7))
                if g == 0:
                    nc.tensor.matmul(sl, st.onesr[:, 0:128],
                                     st.b2_sb[:, cc * 512:(cc + 1) * 512],
                                     start=False, stop=True)
            if g == 0:
                nc.vector.tensor_tensor(y_acc[tt][:], pg[:], st.x2[tt][:],
                                        ALU.add)
            else:
                nc.vector.tensor_tensor(y_acc[tt][:], pg[:], y_acc[tt][:],
                                        ALU.add)
    for tt in range(8):
        nc.sync.dma_start(st.y_h[tt * 128:(tt + 1) * 128, :], y_acc[tt][:])
    utpp.__exit__(None, None, None)
    w2pp.__exit__(None, None, None)
    w1pp.__exit__(None, None, None)
    yacp.__exit__(None, None, None)
    st.h2p.__exit__(None, None, None)
    st.latebp.__exit__(None, None, None)
    st.x2p.__exit__(None, None, None)


def build_program():
    if "nc" in _CACHE:
        return _CACHE["nc"]
    nc = bacc.Bacc(None)
    st = S()

    st.x_h = nc.declare_dram_parameter("x", [T, C], f32, isOutput=False)
    st.xres_h = nc.declare_dram_parameter("xres", [TH, C], f32, isOutput=False)
    st.wq_h = nc.declare_dram_parameter("wq", [C, H], bf16, isOutput=False)
    st.wk_h = nc.declare_dram_parameter("wk", [C, H], bf16, isOutput=False)
    st.wv_h = nc.declare_dram_parameter("wv", [C, H], bf16, isOutput=False)
    bq_h = nc.declare_dram_parameter("bq", [1, H], bf16, isOutput=False)
    bk_h = nc.declare_dram_parameter("bk", [1, H], bf16, isOutput=False)
    bv_h = nc.declare_dram_parameter("bv", [1, H], bf16, isOutput=False)
    st.wo_h = nc.declare_dram_parameter("wo", [C, C], bf16, isOutput=False)
    st.w1_h = nc.declare_dram_parameter("w1", [C, F], bf16, isOutput=False)
    b1_h = nc.declare_dram_parameter("b1", [128, 32], f32, isOutput=False)
    st.w2_h = nc.declare_dram_parameter("w2", [F, C], bf16, isOutput=False)
    b2_h = nc.declare_dram_parameter("b2", [1, C], bf16, isOutput=False)
    ident_h = nc.declare_dram_parameter("ident", [128, 128], bf16, isOutput=False)
    tri_h = nc.declare_dram_parameter("tri", [128, 128], bf16, isOutput=False)
    onesr_h = nc.declare_dram_parameter("onesr", [1, 512], bf16, isOutput=False)
    sel_h = nc.declare_dram_parameter("sel", [128, 2], f32, isOutput=False)
    seln_h = nc.declare_dram_parameter("seln", [128, 2], f32, isOutput=False)
    st.y_h = nc.declare_dram_parameter("y", [TH, C], f32, isOutput=True)

    st.rs_in = [nc.dram_tensor(f"rs_in{hp}", [2, 128, TH], bf16)
                for hp in range(4)]
    st.rs_out = [nc.dram_tensor(f"rs_out{hp}", [128, TH], bf16)
                 for hp in range(4)]

    with tile.TileContext(nc) as tc, ExitStack() as stack:
        st.tc, st.stack = tc, stack
        cst = stack.enter_context(tc.tile_pool(name="const", bufs=1))
        ps = stack.enter_context(tc.tile_pool(name="ps", bufs=1, space="PSUM"))
        st.work = stack.enter_context(tc.tile_pool(name="work", bufs=4))
        st.ptp = stack.enter_context(tc.tile_pool(name="ptp", bufs=3))

        st.ident = cst.tile([128, 128], bf16, name="ident")
        st.tri = cst.tile([128, 128], bf16, name="tri")
        st.onesr = cst.tile([1, 512], bf16, name="onesr")
        st.bq_sb = cst.tile([1, H], bf16, name="bq_sb")
        st.bk_sb = cst.tile([1, H], bf16, name="bk_sb")
        st.bv_sb = cst.tile([1, H], bf16, name="bv_sb")
        st.b1_sb = cst.tile([128, 32], f32, name="b1_sb")
        st.sel_sb = cst.tile([128, 2], f32, name="sel_sb")
        st.seln_sb = cst.tile([128, 2], f32, name="seln_sb")
        for t_, h_ in [(st.ident, ident_h), (st.tri, tri_h), (st.onesr, onesr_h),
                       (st.bq_sb, bq_h), (st.bk_sb, bk_h),
                       (st.bv_sb, bv_h), (st.b1_sb, b1_h),
                       (st.sel_sb, sel_h), (st.seln_sb, seln_h)]:
            nc.sync.dma_start(t_[:], h_[:])
        st.b2_h = b2_h

        def ps_t(tag, shape=(128, 512), dt=f32):
            if tag == "A":
                assert shape[0] <= 128 and shape[1] <= 1024
                full = ps.tile([128, 1024], dt, tag="A", name="ps_A", bufs=2)
                return full[0:shape[0], 0:shape[1]]
            tag = "B" if tag == "pv" else tag
            return ps.tile(list(shape), dt, tag="B", name="ps_B", bufs=4)
        st.ps_t = ps_t

        _phase_hcol(nc, st)
        _phase_attention(nc, st)
        _phase_proj(nc, st)
        _phase_ln2(nc, st)
        _phase_ffn(nc, st)

    nc.compile()
    _CACHE["nc"] = nc
    return nc


def make_inputs(x, Wq, Wk, Wv, Wo, bo, W1, b1, W2, b2,
                ln1_g, ln1_b, ln2_g, ln2_b):
    """Build per-core input maps (host-side sharding + LN folding)."""
    x = np.asarray(x, np.float32)
    scale = float(C) ** -0.5

    wq_eff = ln1_g[:, None] * Wq
    wk_eff = ln1_g[:, None] * Wk * scale
    wv_eff = ln1_g[:, None] * Wv
    bq_full = ln1_b @ Wq
    bk_full = (ln1_b @ Wk) * scale
    bv_full = ln1_b @ Wv
    w1_eff = ln2_g[:, None] * W1
    b1_eff = b1 + ln2_b @ W1

    BF = ml_dtypes.bfloat16
    ident = np.eye(128, dtype=BF)
    tri = np.triu(np.ones((128, 128), BF))
    onesr = np.ones((1, 512), BF)

    in_maps = []
    for core in range(8):
        b, s = core // 2, core % 2
        cs = slice(s * H, (s + 1) * H)
        ts = slice(s * TH, (s + 1) * TH)
        own = np.arange(s * H, (s + 1) * H)
        other = np.arange((1 - s) * H, (2 - s) * H)
        perm = np.concatenate([own, other])
        in_maps.append({
            "x": np.ascontiguousarray(x[b]),
            "xres": np.ascontiguousarray(x[b, ts, :] + bo[None, :]),
            "wq": np.ascontiguousarray(wq_eff[:, cs].astype(BF)),
            "wk": np.ascontiguousarray(wk_eff[:, cs].astype(BF)),
            "wv": np.ascontiguousarray(wv_eff[:, cs].astype(BF)),
            "bq": np.ascontiguousarray(bq_full[cs].reshape(1, H).astype(BF)),
            "bk": np.ascontiguousarray(bk_full[cs].reshape(1, H).astype(BF)),
            "bv": np.ascontiguousarray(bv_full[cs].reshape(1, H).astype(BF)),
            "wo": np.ascontiguousarray(Wo[perm, :].astype(BF)),
            "w1": np.ascontiguousarray(w1_eff.astype(BF)),
            "b1": np.ascontiguousarray(b1_eff.reshape(32, 128).T),
            "w2": np.ascontiguousarray(W2.astype(BF)),
            "b2": np.ascontiguousarray(b2.reshape(1, C).astype(BF)),
            "ident": ident, "tri": tri, "onesr": onesr,
            "sel": np.tile(np.eye(2, dtype=np.float32)[s][None, :], (128, 1)),
            "seln": np.tile(np.eye(2, dtype=np.float32)[1 - s][None, :], (128, 1)),
        })
    return in_maps


def kernel(**inputs):
    nc = build_program()
    in_maps = make_inputs(**{k: np.asarray(v, np.float32) for k, v in inputs.items()})
    res = run_bass_kernel_spmd(nc, in_maps, list(range(8)))
    out = np.empty((B, T, C), np.float32)
    for core in range(8):
        b, s = core // 2, core % 2
        out[b, s * TH:(s + 1) * TH, :] = res.results[core]["y"]
    return out


# revision 8
# speedup vs baseline: 1.0601x; 1.0601x over previous
"""Trainium2 Bass kernel for a dense transformer block (B=4, T=2048, C=1024, 16 heads).

Sharding over 8 NeuronCores: core i handles batch b=i//2 with shard s=i%2.
 - LN1 + QKV + causal attention for its 8 heads (c-slice [512s, 512s+512)) over full T
 - exchange of attention outputs within the (b) pair via 4 halved
   ReduceScatter ops (zero-region trick, fully SPMD-symmetric)
 - proj + LN2 + FFN + residuals on its t-half rows [1024s, 1024s+1024)

All GEMMs run in bf16 with fp32 PSUM accumulation. LayerNorm gain/bias are
folded into the weight matrices on the host; LN statistics use bn_stats and
rsqrt is sqrt(1/(var+eps)) with the reciprocal on the vector engine, so
the ACT table sets never thrash mid-phase.

Scheduling notes (engine queues are strict FIFO, so emission order matters):
 - PSUM uses two shared tags: "A" = 3x [128,1024] slots (GEMM-chain
   accumulators, QK scores) and "B" = 2x [128,512] slots (PE transposes, PV
   accumulators). Chains write both 512-wide halves of an "A" slot and are
   evacuated by a single wide ACT/DVE op.
 - q/k GEMM chains for head-pair hp+1 are emitted inside head-pair hp's
   attention loop, filling PE bubbles while the scalar engine runs exp().
 - The attention kk loop is software-pipelined (PV one step behind QK).
 - proj leaves the RS3 head-pair contribution to a second pass interleaved
   with LN2, so the PE never waits on the last ReduceScatter.
"""

from contextlib import ExitStack

import ml_dtypes
import numpy as np

import concourse.bass as bass
import concourse.mybir as mybir
import concourse.tile as tile
from concourse import bacc
from concourse.bass_utils import run_bass_kernel_spmd

f32 = mybir.dt.float32
bf16 = mybir.dt.bfloat16
AF = mybir.ActivationFunctionType
ALU = mybir.AluOpType
AX = mybir.AxisListType

B, T, C = 4, 2048, 1024
NH, D = 16, 64
F = 4 * C
H = C // 2            # per-core head c-slice (8 heads)
TH = T // 2           # per-core t-half for proj/FFN
EPS = 1e-5
RG = [[0, 1], [2, 3], [4, 5], [6, 7]]

_CACHE = {}


class S:
    """Shared build state."""
    pass


def _layernorm_tile(nc, st, xt, dst):
    """Row-standardize xt [128, C] -> dst [128, C] (bn_stats + ln/exp rsqrt)."""
    work = st.work
    st6 = work.tile([128, 2, 6], f32, name="st6", tag="st6")
    xg = xt[:].rearrange("p (g n) -> p g n", g=2)
    nc.vector.bn_stats(st6[:, 0, :], xg[:, 0, :])
    nc.vector.bn_stats(st6[:, 1, :], xg[:, 1, :])
    mv = work.tile([128, 2], f32, name="mv", tag="mv")
    nc.vector.bn_aggr(mv[:], st6[:])
    veps = work.tile([128, 1], f32, name="veps", tag="veps")
    nc.vector.tensor_scalar_add(veps[:], mv[:, 1:2], EPS)
    riv = work.tile([128, 1], f32, name="riv", tag="riv")
    with nc.allow_low_precision(reason="LN rsqrt"):
        nc.vector.reciprocal(riv[:], veps[:])
    rsig = work.tile([128, 1], f32, name="rsig", tag="rsig")
    nc.scalar.activation(rsig[:], riv[:], AF.Sqrt)
    nmu = work.tile([128, 1], f32, name="nmu", tag="nmu")
    nc.vector.tensor_tensor(nmu[:], mv[:, 0:1], rsig[:], ALU.mult)
    nc.vector.tensor_scalar_mul(nmu[:], nmu[:], -1.0)
    nc.scalar.activation(dst[:], xt[:], AF.Identity, bias=nmu[:], scale=rsig[:])


def _phase_hcol(nc, st):
    """LN1 + transpose (hcol = h^T for all 4 chunks) and the v GEMMs."""
    ps_t = st.ps_t
    st.qkvp = st.tc.tile_pool(name="qkv", bufs=1)
    qkv = st.qkvp.__enter__()
    st.wqkvp = st.tc.tile_pool(name="wqkv", bufs=1)
    wqkv = st.wqkvp.__enter__()
    st.htcp = st.tc.tile_pool(name="htc", bufs=1)
    htc = st.htcp.__enter__()
    st.xhp = st.tc.tile_pool(name="xh", bufs=4)
    xh = st.xhp.__enter__()

    st.wq_sb = [wqkv.tile([128, H], bf16, name=f"wq{k}", tag=f"wq{k}")
                for k in range(8)]
    st.wk_sb = [wqkv.tile([128, H], bf16, name=f"wk{k}", tag=f"wk{k}")
                for k in range(8)]
    wv_sb = [wqkv.tile([128, H], bf16, name=f"wv{k}", tag=f"wv{k}")
             for k in range(8)]
    for k in range(8):
        nc.gpsimd.dma_start(st.wq_sb[k][:], st.wq_h[k * 128:(k + 1) * 128, :])
        nc.gpsimd.dma_start(st.wk_sb[k][:], st.wk_h[k * 128:(k + 1) * 128, :])
        nc.gpsimd.dma_start(wv_sb[k][:], st.wv_h[k * 128:(k + 1) * 128, :])

    st.qT = [qkv.tile([128, T], bf16, name=f"qT{i}", tag=f"qT{i}") for i in range(4)]
    st.kT = [qkv.tile([128, T], bf16, name=f"kT{i}", tag=f"kT{i}") for i in range(4)]
    st.vn = [qkv.tile([128, 520], bf16, name=f"vn{i}", tag=f"vn{i}")
             for i in range(16)]
    st.hcol = [htc.tile([128, 8 * 512], bf16, name=f"hcol{j}", tag=f"hcol{j}")
               for j in range(4)]

    def emit_vchunk(j):
        # v GEMM for chunk j (natural layout, strided into vn + ones col)
        for tt4 in range(4):
            tt = j * 4 + tt4
            pg = ps_t("A")
            for k in range(8):
                nc.tensor.matmul(
                    pg[:],
                    st.hcol[j][:, k * 512 + tt4 * 128:k * 512 + (tt4 + 1) * 128],
                    wv_sb[k][:], start=(k == 0), stop=False)
            nc.tensor.matmul(pg[:], st.onesr[:, 0:128], st.bv_sb[:],
                             start=False, stop=True)
            nc.scalar.copy(
                st.vn[tt][:, 0:520].rearrange("p (h e) -> p h e", h=8)[:, :, 0:64],
                pg[:].rearrange("p (h d) -> p h d", h=8))
            nc.vector.memset(
                st.vn[tt][:, 0:520].rearrange("p (h e) -> p h e", h=8)[:, :, 64:65],
                1.0)

    for j in range(4):  # t-chunks of 512
        for tt4 in range(4):  # t-tiles of 128 within the chunk
            tt = j * 4 + tt4
            xt = xh.tile([128, C], f32, name="xt", tag="xt")
            nc.sync.dma_start(xt[:], st.x_h[tt * 128:(tt + 1) * 128, :])
            ht = xh.tile([128, C], bf16, name="ht", tag="ht")
            _layernorm_tile(nc, st, xt, ht)
            for cc in range(8):
                ptr = ps_t("B", (128, 128), bf16)
                nc.tensor.transpose(ptr[:], ht[:, cc * 128:(cc + 1) * 128],
                                    st.ident[:])
                nc.vector.tensor_copy(
                    out=st.hcol[j][:, cc * 512 + tt4 * 128:
                                   cc * 512 + (tt4 + 1) * 128],
                    in_=ptr[:])
        if j >= 1:
            emit_vchunk(j - 1)
    emit_vchunk(3)
    st.xhp.__exit__(None, None, None)


def _emit_qk_half(nc, st, hp, j, half):
    """q (half=0) or k (half=1) GEMM chain for (head-pair hp, t-chunk j)."""
    dstT, wsb, brow = (((st.qT, st.wq_sb, st.bq_sb),
                        (st.kT, st.wk_sb, st.bk_sb))[half])
    sl = st.ps_t("A")
    for k in range(8):
        nc.tensor.matmul(sl, wsb[k][:, hp * 128:(hp + 1) * 128],
                         st.hcol[j][:, k * 512:(k + 1) * 512],
                         start=(k == 0), stop=False)
    nc.tensor.matmul(sl, brow[0:1, hp * 128:(hp + 1) * 128],
                     st.onesr[:, 0:512], start=False, stop=True)
    nc.vector.tensor_copy(out=dstT[hp][:, j * 512:(j + 1) * 512], in_=sl)


def _phase_attention(nc, st):
    """Causal attention per head-pair; ships results via ReduceScatter.

    The kk loop is software-pipelined: PV(kk) is emitted after QK(kk+1), and
    the QK score PSUM tag is multi-buffered, so the PE streams QK matmuls
    while the scalar engine exponentiates the previous chunk. q/k GEMMs for
    the NEXT head-pair are interleaved at chunk boundaries.
    """
    ps_t = st.ps_t
    st.wop = st.tc.tile_pool(name="wop", bufs=1, side="right")
    wop = st.wop.__enter__()
    st.attp = st.tc.tile_pool(name="attp", bufs=1, side="right")
    attp = st.attp.__enter__()
    st.attsbp = st.tc.tile_pool(name="attsb", bufs=1, side="right")
    attsb = st.attsbp.__enter__()

    attA = [attp.tile([128, T], bf16, name=f"attA{i}", tag=f"attA{i}")
            for i in range(4)]
    st.attA = attA
    st.att_sb = [attsb.tile([128, TH], bf16, name=f"asb{k}", tag=f"asb{k}")
                 for k in range(4)]
    aw = st.tc.tile_pool(name="aw", bufs=2)
    st.awp = aw
    aw = aw.__enter__()
    st.wo_sb = [wop.tile([128, C], bf16, name=f"wo{k}", tag=f"wo{k}")
                for k in range(8)]
    for k in range(8):
        nc.gpsimd.dma_start(st.wo_sb[k][:], st.wo_h[k * 128:(k + 1) * 128, :])

    def emit_pv(hp, kk, nk, r, ptb, po):
        for bi in range(2):
            h = 2 * hp + bi
            nc.tensor.matmul(
                po[bi][0:65, r:512],
                st.vn[kk][:, 65 * h:65 * h + 65],
                ptb[:, bi * 512 + r:bi * 512 + 512],
                start=(kk == 0), stop=(kk == nk - 1))

    def emit_norm(hp, j, po):
        tq0 = j * 512
        sj = j // 2
        for bi, b0 in enumerate((0, 64)):
            rs_row = aw.tile([1, 512], bf16, name="rs_row", tag="rsrow")
            nc.scalar.copy(rs_row[:], po[bi][64:65, :])
            pb = ps_t("A", (64, 512))
            nc.tensor.matmul(pb[:], st.onesr[:, 0:64], rs_row[:],
                             start=True, stop=True)
            rbi = aw.tile([64, 512], f32, name="rbi", tag="rbi")
            nc.vector.reciprocal_approx_fast(rbi[:], pb[:])
            attF = aw.tile([64, 512], bf16, name="attF", tag="attF")
            nc.vector.tensor_tensor(attF[:], po[bi][0:64, :], rbi[:],
                                    ALU.mult)
            nc.vector.tensor_scalar_mul(
                attA[hp][b0:b0 + 64, tq0:tq0 + 512], attF[:],
                st.sel_sb[0:64, sj:sj + 1])
            attBc = aw.tile([64, 512], bf16, name="attBc", tag="attBc")
            nc.vector.tensor_scalar_mul(attBc[:], attF[:],
                                        st.seln_sb[0:64, sj:sj + 1])
            nc.sync.dma_start(
                st.rs_in[hp][sj, b0:b0 + 64,
                             (j % 2) * 512:(j % 2) * 512 + 512],
                attBc[:])

    for j in range(4):
        _emit_qk_half(nc, st, 0, j, 0)
        _emit_qk_half(nc, st, 0, j, 1)
    for hp in range(4):
        norm_pending = None
        for j in range(4):
            tq0 = j * 512
            nk = 4 * (j + 1)
            po = [ps_t("pv"), ps_t("pv")]
            pending = None
            for kk in range(nk):
                r = 128 * (kk - 4 * j) if kk >= 4 * j else 0
                pqk = ps_t("A", (128, 1024))
                for bi, b0 in enumerate((0, 64)):
                    nc.tensor.matmul(
                        pqk[:, bi * 512 + r:bi * 512 + 512],
                        st.kT[hp][b0:b0 + 64, kk * 128:(kk + 1) * 128],
                        st.qT[hp][b0:b0 + 64, tq0 + r:tq0 + 512],
                        start=True, stop=True)
                ptb = st.ptp.tile([128, 1024], bf16, name="ptb", tag="pt")
                if r == 0:
                    nc.scalar.activation(ptb[:], pqk[:], AF.Exp)
                else:
                    nc.scalar.activation(
                        ptb[:].rearrange("p (b w) -> p b w", b=2)[:, :, r:512],
                        pqk[:].rearrange("p (b w) -> p b w", b=2)[:, :, r:512],
                        AF.Exp)
                if kk >= 4 * j:
                    nc.vector.tensor_tensor(
                        ptb[:].rearrange("p (b w) -> p b w", b=2)[:, :, r:r + 128],
                        ptb[:].rearrange("p (b w) -> p b w", b=2)[:, :, r:r + 128],
                        st.tri[:, None, :].to_broadcast((128, 2, 128)),
                        ALU.mult)
                if pending is not None:
                    emit_pv(hp, *pending)
                pending = (kk, nk, r, ptb, po)
                if kk == 1 and hp < 3 and j > 0:
                    _emit_qk_half(nc, st, hp + 1, j - 1, 1)
                if kk == 3 and norm_pending is not None:
                    emit_norm(hp, j - 1, norm_pending)
                    norm_pending = None
            emit_pv(hp, *pending)
            norm_pending = po
            if hp < 3:
                _emit_qk_half(nc, st, hp + 1, j, 0)
                if j == 3:
                    _emit_qk_half(nc, st, hp + 1, 3, 1)
        emit_norm(hp, 3, norm_pending)

        nc.gpsimd.collective_compute(
            "ReduceScatter", ALU.add, replica_groups=RG,
            ins=[st.rs_in[hp][:]], outs=[st.rs_out[hp][:]])
        nc.gpsimd.dma_start(st.att_sb[hp][:], st.rs_out[hp][:])


def _phase_proj(nc, st):
    """Projection pass A: local heads + first three exchanged head-pairs +
    residual. The RS3 contribution is added during LN2 (pass B)."""
    ps_t = st.ps_t
    st.awp.__exit__(None, None, None)
    st.htcp.__exit__(None, None, None)
    st.wqkvp.__exit__(None, None, None)
    st.qkvp.__exit__(None, None, None)
    st.x2p = st.tc.tile_pool(name="x2p", bufs=1)
    x2p = st.x2p.__enter__()
    st.latebp = st.tc.tile_pool(name="lateb", bufs=1)
    lateb = st.latebp.__enter__()
    st.xrpp = st.tc.tile_pool(name="xrp", bufs=2)
    xrp = st.xrpp.__enter__()

    st.b2_sb = lateb.tile([1, C], bf16, name="b2_sb")
    nc.sync.dma_start(st.b2_sb[:], st.b2_h[:])
    st.x2 = [x2p.tile([128, C], f32, name=f"x2_{t}", tag=f"x2_{t}")
             for t in range(8)]
    for tt in range(8):
        xr = xrp.tile([128, C], f32, name="xr", tag="xr")
        nc.sync.dma_start(xr[:], st.xres_h[tt * 128:(tt + 1) * 128, :])
        pg = ps_t("A", (128, 1024))
        for cc in range(2):
            sl = pg[:, cc * 512:(cc + 1) * 512]
            for k in range(4):
                for half in range(2):
                    nc.tensor.matmul(
                        sl,
                        st.attA[k][:, half * TH + tt * 128:
                                   half * TH + (tt + 1) * 128],
                        st.wo_sb[k][:, cc * 512:(cc + 1) * 512],
                        start=(k == 0 and half == 0), stop=False)
            for k in range(3):
                nc.tensor.matmul(sl, st.att_sb[k][:, tt * 128:(tt + 1) * 128],
                                 st.wo_sb[4 + k][:, cc * 512:(cc + 1) * 512],
                                 start=False, stop=(k == 2))
        nc.vector.tensor_tensor(st.x2[tt][:], pg[:], xr[:], ALU.add)
    st.xrpp.__exit__(None, None, None)


def _phase_ln2(nc, st):
    """proj pass B (RS3 head-pair) + LN2 + transpose to h2T, interleaved."""
    ps_t = st.ps_t
    st.h2p = st.tc.tile_pool(name="h2p", bufs=1)
    h2p = st.h2p.__enter__()
    st.h2wp = st.tc.tile_pool(name="h2w", bufs=3)
    h2w = st.h2wp.__enter__()
    st.h2T = [h2p.tile([128, TH], bf16, name=f"h2T{k}", tag=f"h2T{k}")
              for k in range(8)]
    for tt in range(8):
        pgb = ps_t("A", (128, 1024))
        for cc in range(2):
            nc.tensor.matmul(pgb[:, cc * 512:(cc + 1) * 512],
                             st.att_sb[3][:, tt * 128:(tt + 1) * 128],
                             st.wo_sb[7][:, cc * 512:(cc + 1) * 512],
                             start=True, stop=True)
        nc.vector.tensor_tensor(st.x2[tt][:], pgb[:], st.x2[tt][:], ALU.add)
        h2t = h2w.tile([128, C], bf16, name="h2t", tag="h2t")
        _layernorm_tile(nc, st, st.x2[tt], h2t)
        for cc in range(8):
            ptr = ps_t("B", (128, 128), bf16)
            nc.tensor.transpose(ptr[:], h2t[:, cc * 128:(cc + 1) * 128],
                                st.ident[:])
            nc.vector.tensor_copy(out=st.h2T[cc][:, tt * 128:(tt + 1) * 128],
                                  in_=ptr[:])
    st.h2wp.__exit__(None, None, None)
    st.attsbp.__exit__(None, None, None)
    st.attp.__exit__(None, None, None)
    st.wop.__exit__(None, None, None)


def _phase_ffn(nc, st):
    """FFN with grouped ff-dim accumulation, residual, output DMA."""
    ps_t = st.ps_t
    yacp = st.tc.tile_pool(name="yac", bufs=1)
    yac = yacp.__enter__()
    w1pp = st.tc.tile_pool(name="w1p", bufs=16)
    w1p = w1pp.__enter__()
    w2pp = st.tc.tile_pool(name="w2p", bufs=8)
    w2p = w2pp.__enter__()
    utpp = st.tc.tile_pool(name="utp", bufs=8)
    utp = utpp.__enter__()

    y_acc = [yac.tile([128, C], f32, name=f"ya{t}", tag=f"ya{t}")
             for t in range(8)]
    for g in range(4):
        w1g = []
        for k in range(8):
            w1k = w1p.tile([128, 1024], bf16, name="w1k", tag="w1k")
            nc.sync.dma_start(w1k[:],
                              st.w1_h[k * 128:(k + 1) * 128,
                                      g * 1024:(g + 1) * 1024])
            w1g.append(w1k)
        ut_g = []
        for ff in range(8):
            f = g * 8 + ff
            ut = utp.tile([128, TH], bf16, name="ut", tag="ut")
            pg = ps_t("A", (128, 1024))
            for tch in range(2):
                sl = pg[:, tch * 512:(tch + 1) * 512]
                for k in range(8):
                    nc.tensor.matmul(sl, w1g[k][:, ff * 128:(ff + 1) * 128],
                                     st.h2T[k][:, tch * 512:(tch + 1) * 512],
                                     start=(k == 0), stop=(k == 7))
            nc.scalar.activation(ut[:], pg[:], AF.Relu,
                                 bias=st.b1_sb[:, f:f + 1])
            ut_g.append(ut)
        w2g = []
        for ff in range(8):
            f = g * 8 + ff
            w2t = w2p.tile([128, C], bf16, name="w2t", tag="w2t")
            nc.sync.dma_start(w2t[:], st.w2_h[f * 128:(f + 1) * 128, :])
            w2g.append(w2t)
        for tt in range(8):
            pg = ps_t("A", (128, 1024))
            for cc in range(2):
                sl = pg[:, cc * 512:(cc + 1) * 512]
                for ff in range(8):
                    nc.tensor.matmul(sl, ut_g[ff][:, tt * 128:(tt + 1) * 128],
                                     w2g[ff][:, cc * 512:(cc + 1) * 512],
                                     start=(ff == 0),
                                     stop=(False if g == 0 else ff == 7))
                if g == 0:
                    nc.tensor.matmul(sl, st.onesr[:, 0:128],
                                     st.b2_sb[:, cc * 512:(cc + 1) * 512],
                                     start=False, stop=True)
            if g == 0:
                nc.vector.tensor_tensor(y_acc[tt][:], pg[:], st.x2[tt][:],
                                        ALU.add)
            else:
                nc.vector.tensor_tensor(y_acc[tt][:], pg[:], y_acc[tt][:],
                                        ALU.add)
    for tt in range(8):
        nc.sync.dma_start(st.y_h[tt * 128:(tt + 1) * 128, :], y_acc[tt][:])
    utpp.__exit__(None, None, None)
    w2pp.__exit__(None, None, None)
    w1pp.__exit__(None, None, None)
    yacp.__exit__(None, None, None)
    st.h2p.__exit__(None, None, None)
    st.latebp.__exit__(None, None, None)
    st.x2p.__exit__(None, None, None)


def build_program():
    if "nc" in _CACHE:
        return _CACHE["nc"]
    nc = bacc.Bacc(None)
    st = S()

    st.x_h = nc.declare_dram_parameter("x", [T, C], f32, isOutput=False)
    st.xres_h = nc.declare_dram_parameter("xres", [TH, C], f32, isOutput=False)
    st.wq_h = nc.declare_dram_parameter("wq", [C, H], bf16, isOutput=False)
    st.wk_h = nc.declare_dram_parameter("wk", [C, H], bf16, isOutput=False)
    st.wv_h = nc.declare_dram_parameter("wv", [C, H], bf16, isOutput=False)
    bq_h = nc.declare_dram_parameter("bq", [1, H], bf16, isOutput=False)
    bk_h = nc.declare_dram_parameter("bk", [1, H], bf16, isOutput=False)
    bv_h = nc.declare_dram_parameter("bv", [1, H], bf16, isOutput=False)
    st.wo_h = nc.declare_dram_parameter("wo", [C, C], bf16, isOutput=False)
    st.w1_h = nc.declare_dram_parameter("w1", [C, F], bf16, isOutput=False)
    b1_h = nc.declare_dram_parameter("b1", [128, 32], f32, isOutput=False)
    st.w2_h = nc.declare_dram_parameter("w2", [F, C], bf16, isOutput=False)
    b2_h = nc.declare_dram_parameter("b2", [1, C], bf16, isOutput=False)
    ident_h = nc.declare_dram_parameter("ident", [128, 128], bf16, isOutput=False)
    tri_h = nc.declare_dram_parameter("tri", [128, 128], bf16, isOutput=False)
    onesr_h = nc.declare_dram_parameter("onesr", [1, 512], bf16, isOutput=False)
    sel_h = nc.declare_dram_parameter("sel", [128, 2], f32, isOutput=False)
    seln_h = nc.declare_dram_parameter("seln", [128, 2], f32, isOutput=False)
    st.y_h = nc.declare_dram_parameter("y", [TH, C], f32, isOutput=True)

    st.rs_in = [nc.dram_tensor(f"rs_in{hp}", [2, 128, TH], bf16)
                for hp in range(4)]
    st.rs_out = [nc.dram_tensor(f"rs_out{hp}", [128, TH], bf16)
                 for hp in range(4)]

    with tile.TileContext(nc) as tc, ExitStack() as stack:
        st.tc, st.stack = tc, stack
        cst = stack.enter_context(tc.tile_pool(name="const", bufs=1))
        ps = stack.enter_context(tc.tile_pool(name="ps", bufs=1, space="PSUM"))
        st.work = stack.enter_context(tc.tile_pool(name="work", bufs=4))
        st.ptp = stack.enter_context(tc.tile_pool(name="ptp", bufs=3))

        st.ident = cst.tile([128, 128], bf16, name="ident")
        st.tri = cst.tile([128, 128], bf16, name="tri")
        st.onesr = cst.tile([1, 512], bf16, name="onesr")
        st.bq_sb = cst.tile([1, H], bf16, name="bq_sb")
        st.bk_sb = cst.tile([1, H], bf16, name="bk_sb")
        st.bv_sb = cst.tile([1, H], bf16, name="bv_sb")
        st.b1_sb = cst.tile([128, 32], f32, name="b1_sb")
        st.sel_sb = cst.tile([128, 2], f32, name="sel_sb")
        st.seln_sb = cst.tile([128, 2], f32, name="seln_sb")
        for t_, h_ in [(st.ident, ident_h), (st.tri, tri_h), (st.onesr, onesr_h),
                       (st.bq_sb, bq_h), (st.bk_sb, bk_h),
                       (st.bv_sb, bv_h), (st.b1_sb, b1_h),
                       (st.sel_sb, sel_h), (st.seln_sb, seln_h)]:
            nc.sync.dma_start(t_[:], h_[:])
        st.b2_h = b2_h

        def ps_t(tag, shape=(128, 512), dt=f32):
            if tag == "A":
                assert shape[0] <= 128 and shape[1] <= 1024
                full = ps.tile([128, 1024], dt, tag="A", name="ps_A", bufs=2)
                return full[0:shape[0], 0:shape[1]]
            tag = "B" if tag == "pv" else tag
            return ps.tile(list(shape), dt, tag="B", name="ps_B", bufs=4)
        st.ps_t = ps_t

        _phase_hcol(nc, st)
        _phase_attention(nc, st)
        _phase_proj(nc, st)
        _phase_ln2(nc, st)
        _phase_ffn(nc, st)

    nc.compile()
    _CACHE["nc"] = nc
    return nc


def make_inputs(x, Wq, Wk, Wv, Wo, bo, W1, b1, W2, b2,
                ln1_g, ln1_b, ln2_g, ln2_b):
    """Build per-core input maps (host-side sharding + LN folding)."""
    x = np.asarray(x, np.float32)
    scale = float(C) ** -0.5

    wq_eff = ln1_g[:, None] * Wq
    wk_eff = ln1_g[:, None] * Wk * scale
    wv_eff = ln1_g[:, None] * Wv
    bq_full = ln1_b @ Wq
    bk_full = (ln1_b @ Wk) * scale
    bv_full = ln1_b @ Wv
    w1_eff = ln2_g[:, None] * W1
    b1_eff = b1 + ln2_b @ W1

    BF = ml_dtypes.bfloat16
    ident = np.eye(128, dtype=BF)
    tri = np.triu(np.ones((128, 128), BF))
    onesr = np.ones((1, 512), BF)

    in_maps = []
    for core in range(8):
        b, s = core // 2, core % 2
        cs = slice(s * H, (s + 1) * H)
        ts = slice(s * TH, (s + 1) * TH)
        own = np.arange(s * H, (s + 1) * H)
        other = np.arange((1 - s) * H, (2 - s) * H)
        perm = np.concatenate([own, other])
        in_maps.append({
            "x": np.ascontiguousarray(x[b]),
            "xres": np.ascontiguousarray(x[b, ts, :] + bo[None, :]),
            "wq": np.ascontiguousarray(wq_eff[:, cs].astype(BF)),
            "wk": np.ascontiguousarray(wk_eff[:, cs].astype(BF)),
            "wv": np.ascontiguousarray(wv_eff[:, cs].astype(BF)),
            "bq": np.ascontiguousarray(bq_full[cs].reshape(1, H).astype(BF)),
            "bk": np.ascontiguousarray(bk_full[cs].reshape(1, H).astype(BF)),
            "bv": np.ascontiguousarray(bv_full[cs].reshape(1, H).astype(BF)),
            "wo": np.ascontiguousarray(Wo[perm, :].astype(BF)),
            "w1": np.ascontiguousarray(w1_eff.astype(BF)),
            "b1": np.ascontiguousarray(b1_eff.reshape(32, 128).T),
            "w2": np.ascontiguousarray(W2.astype(BF)),
            "b2": np.ascontiguousarray(b2.reshape(1, C).astype(BF)),
            "ident": ident, "tri": tri, "onesr": onesr,
            "sel": np.tile(np.eye(2, dtype=np.float32)[s][None, :], (128, 1)),
            "seln": np.tile(np.eye(2, dtype=np.float32)[1 - s][None, :], (128, 1)),
        })
    return in_maps


def kernel(**inputs):
    nc = build_program()
    in_maps = make_inputs(**{k: np.asarray(v, np.float32) for k, v in inputs.items()})
    res = run_bass_kernel_spmd(nc, in_maps, list(range(8)))
    out = np.empty((B, T, C), np.float32)
    for core in range(8):
        b, s = core // 2, core % 2
        out[b, s * TH:(s + 1) * TH, :] = res.results[core]["y"]
    return out


# revision 9
# speedup vs baseline: 1.0721x; 1.0113x over previous
"""Trainium2 Bass kernel for a dense transformer block (B=4, T=2048, C=1024, 16 heads).

Sharding over 8 NeuronCores: core i handles batch b=i//2 with shard s=i%2.
 - LN1 + QKV + causal attention for its 8 heads (c-slice [512s, 512s+512)) over full T
 - exchange of attention outputs within the (b) pair via 4 halved
   ReduceScatter ops (zero-region trick, fully SPMD-symmetric)
 - proj + LN2 + FFN + residuals on its t-half rows [1024s, 1024s+1024)

All GEMMs run in bf16 with fp32 PSUM accumulation. LayerNorm gain/bias are
folded into the weight matrices on the host; LN statistics use bn_stats and
rsqrt is sqrt(1/(var+eps)) with the reciprocal on the vector engine, so
the ACT table sets never thrash mid-phase.

Scheduling notes (engine queues are strict FIFO, so emission order matters):
 - PSUM uses two shared tags: "A" = 3x [128,1024] slots (GEMM-chain
   accumulators, QK scores) and "B" = 2x [128,512] slots (PE transposes, PV
   accumulators). Chains write both 512-wide halves of an "A" slot and are
   evacuated by a single wide ACT/DVE op.
 - q/k GEMM chains for head-pair hp+1 are emitted inside head-pair hp's
   attention loop, filling PE bubbles while the scalar engine runs exp().
 - The attention kk loop is software-pipelined (PV one step behind QK).
 - proj leaves the RS3 head-pair contribution to a second pass interleaved
   with LN2, so the PE never waits on the last ReduceScatter.
"""

from contextlib import ExitStack

import ml_dtypes
import numpy as np

import concourse.bass as bass
import concourse.mybir as mybir
import concourse.tile as tile
from concourse import bacc
from concourse.bass_utils import run_bass_kernel_spmd

f32 = mybir.dt.float32
bf16 = mybir.dt.bfloat16
AF = mybir.ActivationFunctionType
ALU = mybir.AluOpType
AX = mybir.AxisListType

B, T, C = 4, 2048, 1024
NH, D = 16, 64
F = 4 * C
H = C // 2            # per-core head c-slice (8 heads)
TH = T // 2           # per-core t-half for proj/FFN
EPS = 1e-5
RG = [[0, 1], [2, 3], [4, 5], [6, 7]]

_CACHE = {}


class S:
    """Shared build state."""
    pass


def _layernorm_tile(nc, st, xt, dst):
    """Row-standardize xt [128, C] -> dst [128, C] (bn_stats + ln/exp rsqrt)."""
    work = st.work
    st6 = work.tile([128, 2, 6], f32, name="st6", tag="st6")
    xg = xt[:].rearrange("p (g n) -> p g n", g=2)
    nc.vector.bn_stats(st6[:, 0, :], xg[:, 0, :])
    nc.vector.bn_stats(st6[:, 1, :], xg[:, 1, :])
    mv = work.tile([128, 2], f32, name="mv", tag="mv")
    nc.vector.bn_aggr(mv[:], st6[:])
    veps = work.tile([128, 1], f32, name="veps", tag="veps")
    nc.vector.tensor_scalar_add(veps[:], mv[:, 1:2], EPS)
    riv = work.tile([128, 1], f32, name="riv", tag="riv")
    with nc.allow_low_precision(reason="LN rsqrt"):
        nc.vector.reciprocal(riv[:], veps[:])
    rsig = work.tile([128, 1], f32, name="rsig", tag="rsig")
    nc.scalar.activation(rsig[:], riv[:], AF.Sqrt)
    nmu = work.tile([128, 1], f32, name="nmu", tag="nmu")
    nc.vector.tensor_tensor(nmu[:], mv[:, 0:1], rsig[:], ALU.mult)
    nc.vector.tensor_scalar_mul(nmu[:], nmu[:], -1.0)
    nc.scalar.activation(dst[:], xt[:], AF.Identity, bias=nmu[:], scale=rsig[:])


def _phase_hcol(nc, st):
    """LN1 + transpose (hcol = h^T for all 4 chunks) and the v GEMMs."""
    ps_t = st.ps_t
    st.qkvp = st.tc.tile_pool(name="qkv", bufs=1)
    qkv = st.qkvp.__enter__()
    st.wqkvp = st.tc.tile_pool(name="wqkv", bufs=1)
    wqkv = st.wqkvp.__enter__()
    st.htcp = st.tc.tile_pool(name="htc", bufs=1)
    htc = st.htcp.__enter__()
    st.xhp = st.tc.tile_pool(name="xh", bufs=4)
    xh = st.xhp.__enter__()

    st.wq_sb = [wqkv.tile([128, H], bf16, name=f"wq{k}", tag=f"wq{k}")
                for k in range(8)]
    st.wk_sb = [wqkv.tile([128, H], bf16, name=f"wk{k}", tag=f"wk{k}")
                for k in range(8)]
    wv_sb = [wqkv.tile([128, H], bf16, name=f"wv{k}", tag=f"wv{k}")
             for k in range(8)]
    for k in range(8):
        nc.gpsimd.dma_start(st.wq_sb[k][:], st.wq_h[k * 128:(k + 1) * 128, :])
        nc.gpsimd.dma_start(st.wk_sb[k][:], st.wk_h[k * 128:(k + 1) * 128, :])
        nc.gpsimd.dma_start(wv_sb[k][:], st.wv_h[k * 128:(k + 1) * 128, :])

    st.qT = [qkv.tile([128, T], bf16, name=f"qT{i}", tag=f"qT{i}") for i in range(4)]
    st.kT = [qkv.tile([128, T], bf16, name=f"kT{i}", tag=f"kT{i}") for i in range(4)]
    st.vn = [qkv.tile([128, 520], bf16, name=f"vn{i}", tag=f"vn{i}")
             for i in range(16)]
    st.hcol = [htc.tile([128, 8 * 512], bf16, name=f"hcol{j}", tag=f"hcol{j}")
               for j in range(4)]

    def emit_vchunk(j):
        # v GEMM for chunk j (natural layout, strided into vn + ones col)
        for tt4 in range(4):
            tt = j * 4 + tt4
            pg = ps_t("A")
            for k in range(8):
                nc.tensor.matmul(
                    pg[:],
                    st.hcol[j][:, k * 512 + tt4 * 128:k * 512 + (tt4 + 1) * 128],
                    wv_sb[k][:], start=(k == 0), stop=False)
            nc.tensor.matmul(pg[:], st.onesr[:, 0:128], st.bv_sb[:],
                             start=False, stop=True)
            nc.scalar.copy(
                st.vn[tt][:, 0:520].rearrange("p (h e) -> p h e", h=8)[:, :, 0:64],
                pg[:].rearrange("p (h d) -> p h d", h=8))
            nc.vector.memset(
                st.vn[tt][:, 0:520].rearrange("p (h e) -> p h e", h=8)[:, :, 64:65],
                1.0)

    for j in range(4):  # t-chunks of 512
        for tt4 in range(4):  # t-tiles of 128 within the chunk
            tt = j * 4 + tt4
            xt = xh.tile([128, C], f32, name="xt", tag="xt")
            nc.sync.dma_start(xt[:], st.x_h[tt * 128:(tt + 1) * 128, :])
            ht = xh.tile([128, C], bf16, name="ht", tag="ht")
            _layernorm_tile(nc, st, xt, ht)
            for cc in range(8):
                ptr = ps_t("B", (128, 128), bf16)
                nc.tensor.transpose(ptr[:], ht[:, cc * 128:(cc + 1) * 128],
                                    st.ident[:])
                dst = st.hcol[j][:, cc * 512 + tt4 * 128:
                                cc * 512 + (tt4 + 1) * 128]
                if cc < 3:
                    nc.scalar.copy(dst, ptr[:])
                else:
                    nc.vector.tensor_copy(out=dst, in_=ptr[:])
        if j >= 1:
            emit_vchunk(j - 1)
    emit_vchunk(3)
    st.xhp.__exit__(None, None, None)


def _emit_qk_half(nc, st, hp, j, half):
    """q (half=0) or k (half=1) GEMM chain for (head-pair hp, t-chunk j)."""
    dstT, wsb, brow = (((st.qT, st.wq_sb, st.bq_sb),
                        (st.kT, st.wk_sb, st.bk_sb))[half])
    sl = st.ps_t("A")
    for k in range(8):
        nc.tensor.matmul(sl, wsb[k][:, hp * 128:(hp + 1) * 128],
                         st.hcol[j][:, k * 512:(k + 1) * 512],
                         start=(k == 0), stop=False)
    nc.tensor.matmul(sl, brow[0:1, hp * 128:(hp + 1) * 128],
                     st.onesr[:, 0:512], start=False, stop=True)
    nc.vector.tensor_copy(out=dstT[hp][:, j * 512:(j + 1) * 512], in_=sl)


def _phase_attention(nc, st):
    """Causal attention per head-pair; ships results via ReduceScatter.

    The kk loop is software-pipelined: PV(kk) is emitted after QK(kk+1), and
    the QK score PSUM tag is multi-buffered, so the PE streams QK matmuls
    while the scalar engine exponentiates the previous chunk. q/k GEMMs for
    the NEXT head-pair are interleaved at chunk boundaries.
    """
    ps_t = st.ps_t
    st.wop = st.tc.tile_pool(name="wop", bufs=1, side="right")
    wop = st.wop.__enter__()
    st.attp = st.tc.tile_pool(name="attp", bufs=1, side="right")
    attp = st.attp.__enter__()
    st.attsbp = st.tc.tile_pool(name="attsb", bufs=1, side="right")
    attsb = st.attsbp.__enter__()

    attA = [attp.tile([128, T], bf16, name=f"attA{i}", tag=f"attA{i}")
            for i in range(4)]
    st.attA = attA
    st.att_sb = [attsb.tile([128, TH], bf16, name=f"asb{k}", tag=f"asb{k}")
                 for k in range(4)]
    aw = st.tc.tile_pool(name="aw", bufs=2)
    st.awp = aw
    aw = aw.__enter__()
    st.wo_sb = [wop.tile([128, C], bf16, name=f"wo{k}", tag=f"wo{k}")
                for k in range(8)]
    for k in range(8):
        nc.gpsimd.dma_start(st.wo_sb[k][:], st.wo_h[k * 128:(k + 1) * 128, :])

    def emit_pv(hp, kk, nk, r, ptb, po):
        for bi in range(2):
            h = 2 * hp + bi
            nc.tensor.matmul(
                po[bi][0:65, r:512],
                st.vn[kk][:, 65 * h:65 * h + 65],
                ptb[:, bi * 512 + r:bi * 512 + 512],
                start=(kk == 0), stop=(kk == nk - 1))

    def emit_norm(hp, j, po):
        tq0 = j * 512
        sj = j // 2
        for bi, b0 in enumerate((0, 64)):
            rs_row = aw.tile([1, 512], bf16, name="rs_row", tag="rsrow")
            nc.vector.tensor_copy(out=rs_row[:], in_=po[bi][64:65, :])
            pb = ps_t("A", (64, 512))
            nc.tensor.matmul(pb[:], st.onesr[:, 0:64], rs_row[:],
                             start=True, stop=True)
            rbi = aw.tile([64, 512], f32, name="rbi", tag="rbi")
            nc.vector.reciprocal_approx_fast(rbi[:], pb[:])
            attF = aw.tile([64, 512], bf16, name="attF", tag="attF")
            nc.vector.tensor_tensor(attF[:], po[bi][0:64, :], rbi[:],
                                    ALU.mult)
            nc.vector.tensor_scalar_mul(
                attA[hp][b0:b0 + 64, tq0:tq0 + 512], attF[:],
                st.sel_sb[0:64, sj:sj + 1])
            attBc = aw.tile([64, 512], bf16, name="attBc", tag="attBc")
            nc.vector.tensor_scalar_mul(attBc[:], attF[:],
                                        st.seln_sb[0:64, sj:sj + 1])
            nc.sync.dma_start(
                st.rs_in[hp][sj, b0:b0 + 64,
                             (j % 2) * 512:(j % 2) * 512 + 512],
                attBc[:])

    for j in range(4):
        _emit_qk_half(nc, st, 0, j, 0)
        _emit_qk_half(nc, st, 0, j, 1)
    for hp in range(4):
        norm_pending = None
        for j in range(4):
            tq0 = j * 512
            nk = 4 * (j + 1)
            po = [ps_t("pv"), ps_t("pv")]
            pending = None
            for kk in range(nk):
                r = 128 * (kk - 4 * j) if kk >= 4 * j else 0
                pqk = ps_t("A", (128, 1024))
                for bi, b0 in enumerate((0, 64)):
                    nc.tensor.matmul(
                        pqk[:, bi * 512 + r:bi * 512 + 512],
                        st.kT[hp][b0:b0 + 64, kk * 128:(kk + 1) * 128],
                        st.qT[hp][b0:b0 + 64, tq0 + r:tq0 + 512],
                        start=True, stop=True)
                ptb = st.ptp.tile([128, 1024], bf16, name="ptb", tag="pt")
                if r == 0:
                    nc.scalar.activation(ptb[:], pqk[:], AF.Exp)
                else:
                    nc.scalar.activation(
                        ptb[:].rearrange("p (b w) -> p b w", b=2)[:, :, r:512],
                        pqk[:].rearrange("p (b w) -> p b w", b=2)[:, :, r:512],
                        AF.Exp)
                if kk >= 4 * j:
                    nc.vector.tensor_tensor(
                        ptb[:].rearrange("p (b w) -> p b w", b=2)[:, :, r:r + 128],
                        ptb[:].rearrange("p (b w) -> p b w", b=2)[:, :, r:r + 128],
                        st.tri[:, None, :].to_broadcast((128, 2, 128)),
                        ALU.mult)
                if pending is not None:
                    emit_pv(hp, *pending)
                pending = (kk, nk, r, ptb, po)
                if kk == 1 and hp < 3 and j > 0:
                    _emit_qk_half(nc, st, hp + 1, j - 1, 1)
                if kk == 3 and norm_pending is not None:
                    emit_norm(hp, j - 1, norm_pending)
                    norm_pending = None
            emit_pv(hp, *pending)
            norm_pending = po
            if hp < 3:
                _emit_qk_half(nc, st, hp + 1, j, 0)
                if j == 3:
                    _emit_qk_half(nc, st, hp + 1, 3, 1)
        emit_norm(hp, 3, norm_pending)

        nc.gpsimd.collective_compute(
            "ReduceScatter", ALU.add, replica_groups=RG,
            ins=[st.rs_in[hp][:]], outs=[st.rs_out[hp][:]])
        nc.gpsimd.dma_start(st.att_sb[hp][:], st.rs_out[hp][:])


def _phase_proj(nc, st):
    """Projection pass A: local heads + first three exchanged head-pairs +
    residual. The RS3 contribution is added during LN2 (pass B)."""
    ps_t = st.ps_t
    st.awp.__exit__(None, None, None)
    st.htcp.__exit__(None, None, None)
    st.wqkvp.__exit__(None, None, None)
    st.qkvp.__exit__(None, None, None)
    st.x2p = st.tc.tile_pool(name="x2p", bufs=1)
    x2p = st.x2p.__enter__()
    st.latebp = st.tc.tile_pool(name="lateb", bufs=1)
    lateb = st.latebp.__enter__()
    st.xrpp = st.tc.tile_pool(name="xrp", bufs=2)
    xrp = st.xrpp.__enter__()

    st.b2_sb = lateb.tile([1, C], bf16, name="b2_sb")
    nc.sync.dma_start(st.b2_sb[:], st.b2_h[:])
    st.x2 = [x2p.tile([128, C], f32, name=f"x2_{t}", tag=f"x2_{t}")
             for t in range(8)]
    for tt in range(8):
        xr = xrp.tile([128, C], f32, name="xr", tag="xr")
        nc.sync.dma_start(xr[:], st.xres_h[tt * 128:(tt + 1) * 128, :])
        pg = ps_t("A", (128, 1024))
        for cc in range(2):
            sl = pg[:, cc * 512:(cc + 1) * 512]
            for k in range(4):
                for half in range(2):
                    nc.tensor.matmul(
                        sl,
                        st.attA[k][:, half * TH + tt * 128:
                                   half * TH + (tt + 1) * 128],
                        st.wo_sb[k][:, cc * 512:(cc + 1) * 512],
                        start=(k == 0 and half == 0), stop=False)
            for k in range(3):
                nc.tensor.matmul(sl, st.att_sb[k][:, tt * 128:(tt + 1) * 128],
                                 st.wo_sb[4 + k][:, cc * 512:(cc + 1) * 512],
                                 start=False, stop=(k == 2))
        nc.vector.tensor_tensor(st.x2[tt][:], pg[:], xr[:], ALU.add)
    st.xrpp.__exit__(None, None, None)


def _phase_ln2(nc, st):
    """proj pass B (RS3 head-pair) + LN2 + transpose to h2T, interleaved."""
    ps_t = st.ps_t
    st.h2p = st.tc.tile_pool(name="h2p", bufs=1)
    h2p = st.h2p.__enter__()
    st.h2wp = st.tc.tile_pool(name="h2w", bufs=3)
    h2w = st.h2wp.__enter__()
    st.h2T = [h2p.tile([128, TH], bf16, name=f"h2T{k}", tag=f"h2T{k}")
              for k in range(8)]
    for tt in range(8):
        pgb = ps_t("A", (128, 1024))
        for cc in range(2):
            nc.tensor.matmul(pgb[:, cc * 512:(cc + 1) * 512],
                             st.att_sb[3][:, tt * 128:(tt + 1) * 128],
                             st.wo_sb[7][:, cc * 512:(cc + 1) * 512],
                             start=True, stop=True)
        nc.vector.tensor_tensor(st.x2[tt][:], pgb[:], st.x2[tt][:], ALU.add)
        h2t = h2w.tile([128, C], bf16, name="h2t", tag="h2t")
        _layernorm_tile(nc, st, st.x2[tt], h2t)
        for cc in range(8):
            ptr = ps_t("B", (128, 128), bf16)
            nc.tensor.transpose(ptr[:], h2t[:, cc * 128:(cc + 1) * 128],
                                st.ident[:])
            if cc < 5:
                nc.scalar.copy(st.h2T[cc][:, tt * 128:(tt + 1) * 128], ptr[:])
            else:
                nc.vector.tensor_copy(
                    out=st.h2T[cc][:, tt * 128:(tt + 1) * 128], in_=ptr[:])
    st.h2wp.__exit__(None, None, None)
    st.attsbp.__exit__(None, None, None)
    st.attp.__exit__(None, None, None)
    st.wop.__exit__(None, None, None)


def _phase_ffn(nc, st):
    """FFN with grouped ff-dim accumulation, residual, output DMA."""
    ps_t = st.ps_t
    yacp = st.tc.tile_pool(name="yac", bufs=1)
    yac = yacp.__enter__()
    w1pp = st.tc.tile_pool(name="w1p", bufs=16)
    w1p = w1pp.__enter__()
    w2pp = st.tc.tile_pool(name="w2p", bufs=8)
    w2p = w2pp.__enter__()
    utpp = st.tc.tile_pool(name="utp", bufs=8)
    utp = utpp.__enter__()

    y_acc = [yac.tile([128, C], f32, name=f"ya{t}", tag=f"ya{t}")
             for t in range(8)]
    for g in range(4):
        w1g = []
        for k in range(8):
            w1k = w1p.tile([128, 1024], bf16, name="w1k", tag="w1k")
            nc.sync.dma_start(w1k[:],
                              st.w1_h[k * 128:(k + 1) * 128,
                                      g * 1024:(g + 1) * 1024])
            w1g.append(w1k)
        ut_g = []
        for ff in range(8):
            f = g * 8 + ff
            ut = utp.tile([128, TH], bf16, name="ut", tag="ut")
            pg = ps_t("A", (128, 1024))
            for tch in range(2):
                sl = pg[:, tch * 512:(tch + 1) * 512]
                for k in range(8):
                    nc.tensor.matmul(sl, w1g[k][:, ff * 128:(ff + 1) * 128],
                                     st.h2T[k][:, tch * 512:(tch + 1) * 512],
                                     start=(k == 0), stop=(k == 7))
            nc.scalar.activation(ut[:], pg[:], AF.Relu,
                                 bias=st.b1_sb[:, f:f + 1])
            ut_g.append(ut)
        w2g = []
        for ff in range(8):
            f = g * 8 + ff
            w2t = w2p.tile([128, C], bf16, name="w2t", tag="w2t")
            nc.sync.dma_start(w2t[:], st.w2_h[f * 128:(f + 1) * 128, :])
            w2g.append(w2t)
        for tt in range(8):
            pg = ps_t("A", (128, 1024))
            for cc in range(2):
                sl = pg[:, cc * 512:(cc + 1) * 512]
                for ff in range(8):
                    nc.tensor.matmul(sl, ut_g[ff][:, tt * 128:(tt + 1) * 128],
                                     w2g[ff][:, cc * 512:(cc + 1) * 512],
                                     start=(ff == 0),
                                     stop=(False if g == 0 else ff == 7))
                if g == 0:
                    nc.tensor.matmul(sl, st.onesr[:, 0:128],
                                     st.b2_sb[:, cc * 512:(cc + 1) * 512],
                                     start=False, stop=True)
            if g == 0:
                nc.vector.tensor_tensor(y_acc[tt][:], pg[:], st.x2[tt][:],
                                        ALU.add)
            else:
                nc.vector.tensor_tensor(y_acc[tt][:], pg[:], y_acc[tt][:],
                                        ALU.add)
    for tt in range(8):
        nc.sync.dma_start(st.y_h[tt * 128:(tt + 1) * 128, :], y_acc[tt][:])
    utpp.__exit__(None, None, None)
    w2pp.__exit__(None, None, None)
    w1pp.__exit__(None, None, None)
    yacp.__exit__(None, None, None)
    st.h2p.__exit__(None, None, None)
    st.latebp.__exit__(None, None, None)
    st.x2p.__exit__(None, None, None)


def build_program():
    if "nc" in _CACHE:
        return _CACHE["nc"]
    nc = bacc.Bacc(None)
    st = S()

    st.x_h = nc.declare_dram_parameter("x", [T, C], f32, isOutput=False)
    st.xres_h = nc.declare_dram_parameter("xres", [TH, C], f32, isOutput=False)
    st.wq_h = nc.declare_dram_parameter("wq", [C, H], bf16, isOutput=False)
    st.wk_h = nc.declare_dram_parameter("wk", [C, H], bf16, isOutput=False)
    st.wv_h = nc.declare_dram_parameter("wv", [C, H], bf16, isOutput=False)
    bq_h = nc.declare_dram_parameter("bq", [1, H], bf16, isOutput=False)
    bk_h = nc.declare_dram_parameter("bk", [1, H], bf16, isOutput=False)
    bv_h = nc.declare_dram_parameter("bv", [1, H], bf16, isOutput=False)
    st.wo_h = nc.declare_dram_parameter("wo", [C, C], bf16, isOutput=False)
    st.w1_h = nc.declare_dram_parameter("w1", [C, F], bf16, isOutput=False)
    b1_h = nc.declare_dram_parameter("b1", [128, 32], f32, isOutput=False)
    st.w2_h = nc.declare_dram_parameter("w2", [F, C], bf16, isOutput=False)
    b2_h = nc.declare_dram_parameter("b2", [1, C], bf16, isOutput=False)
    ident_h = nc.declare_dram_parameter("ident", [128, 128], bf16, isOutput=False)
    tri_h = nc.declare_dram_parameter("tri", [128, 128], bf16, isOutput=False)
    onesr_h = nc.declare_dram_parameter("onesr", [1, 512], bf16, isOutput=False)
    sel_h = nc.declare_dram_parameter("sel", [128, 2], f32, isOutput=False)
    seln_h = nc.declare_dram_parameter("seln", [128, 2], f32, isOutput=False)
    st.y_h = nc.declare_dram_parameter("y", [TH, C], f32, isOutput=True)

    st.rs_in = [nc.dram_tensor(f"rs_in{hp}", [2, 128, TH], bf16)
                for hp in range(4)]
    st.rs_out = [nc.dram_tensor(f"rs_out{hp}", [128, TH], bf16)
                 for hp in range(4)]

    with tile.TileContext(nc) as tc, ExitStack() as stack:
        st.tc, st.stack = tc, stack
        cst = stack.enter_context(tc.tile_pool(name="const", bufs=1))
        ps = stack.enter_context(tc.tile_pool(name="ps", bufs=1, space="PSUM"))
        st.work = stack.enter_context(tc.tile_pool(name="work", bufs=4))
        st.ptp = stack.enter_context(tc.tile_pool(name="ptp", bufs=3))

        st.ident = cst.tile([128, 128], bf16, name="ident")
        st.tri = cst.tile([128, 128], bf16, name="tri")
        st.onesr = cst.tile([1, 512], bf16, name="onesr")
        st.bq_sb = cst.tile([1, H], bf16, name="bq_sb")
        st.bk_sb = cst.tile([1, H], bf16, name="bk_sb")
        st.bv_sb = cst.tile([1, H], bf16, name="bv_sb")
        st.b1_sb = cst.tile([128, 32], f32, name="b1_sb")
        st.sel_sb = cst.tile([128, 2], f32, name="sel_sb")
        st.seln_sb = cst.tile([128, 2], f32, name="seln_sb")
        for t_, h_ in [(st.ident, ident_h), (st.tri, tri_h), (st.onesr, onesr_h),
                       (st.bq_sb, bq_h), (st.bk_sb, bk_h),
                       (st.bv_sb, bv_h), (st.b1_sb, b1_h),
                       (st.sel_sb, sel_h), (st.seln_sb, seln_h)]:
            nc.sync.dma_start(t_[:], h_[:])
        st.b2_h = b2_h

        def ps_t(tag, shape=(128, 512), dt=f32):
            if tag == "A":
                assert shape[0] <= 128 and shape[1] <= 1024
                full = ps.tile([128, 1024], dt, tag="A", name="ps_A", bufs=2)
                return full[0:shape[0], 0:shape[1]]
            tag = "B" if tag == "pv" else tag
            return ps.tile(list(shape), dt, tag="B", name="ps_B", bufs=4)
        st.ps_t = ps_t

        _phase_hcol(nc, st)
        _phase_attention(nc, st)
        _phase_proj(nc, st)
        _phase_ln2(nc, st)
        _phase_ffn(nc, st)

    nc.compile()
    _CACHE["nc"] = nc
    return nc


def make_inputs(x, Wq, Wk, Wv, Wo, bo, W1, b1, W2, b2,
                ln1_g, ln1_b, ln2_g, ln2_b):
    """Build per-core input maps (host-side sharding + LN folding)."""
    x = np.asarray(x, np.float32)
    scale = float(C) ** -0.5

    wq_eff = ln1_g[:, None] * Wq
    wk_eff = ln1_g[:, None] * Wk * scale
    wv_eff = ln1_g[:, None] * Wv
    bq_full = ln1_b @ Wq
    bk_full = (ln1_b @ Wk) * scale
    bv_full = ln1_b @ Wv
    w1_eff = ln2_g[:, None] * W1
    b1_eff = b1 + ln2_b @ W1

    BF = ml_dtypes.bfloat16
    ident = np.eye(128, dtype=BF)
    tri = np.triu(np.ones((128, 128), BF))
    onesr = np.ones((1, 512), BF)

    in_maps = []
    for core in range(8):
        b, s = core // 2, core % 2
        cs = slice(s * H, (s + 1) * H)
        ts = slice(s * TH, (s + 1) * TH)
        own = np.arange(s * H, (s + 1) * H)
        other = np.arange((1 - s) * H, (2 - s) * H)
        perm = np.concatenate([own, other])
        in_maps.append({
            "x": np.ascontiguousarray(x[b]),
            "xres": np.ascontiguousarray(x[b, ts, :] + bo[None, :]),
            "wq": np.ascontiguousarray(wq_eff[:, cs].astype(BF)),
            "wk": np.ascontiguousarray(wk_eff[:, cs].astype(BF)),
            "wv": np.ascontiguousarray(wv_eff[:, cs].astype(BF)),
            "bq": np.ascontiguousarray(bq_full[cs].reshape(1, H).astype(BF)),
            "bk": np.ascontiguousarray(bk_full[cs].reshape(1, H).astype(BF)),
            "bv": np.ascontiguousarray(bv_full[cs].reshape(1, H).astype(BF)),
            "wo": np.ascontiguousarray(Wo[perm, :].astype(BF)),
            "w1": np.ascontiguousarray(w1_eff.astype(BF)),
            "b1": np.ascontiguousarray(b1_eff.reshape(32, 128).T),
            "w2": np.ascontiguousarray(W2.astype(BF)),
            "b2": np.ascontiguousarray(b2.reshape(1, C).astype(BF)),
            "ident": ident, "tri": tri, "onesr": onesr,
            "sel": np.tile(np.eye(2, dtype=np.float32)[s][None, :], (128, 1)),
            "seln": np.tile(np.eye(2, dtype=np.float32)[1 - s][None, :], (128, 1)),
        })
    return in_maps


def kernel(**inputs):
    nc = build_program()
    in_maps = make_inputs(**{k: np.asarray(v, np.float32) for k, v in inputs.items()})
    res = run_bass_kernel_spmd(nc, in_maps, list(range(8)))
    out = np.empty((B, T, C), np.float32)
    for core in range(8):
        b, s = core // 2, core % 2
        out[b, s * TH:(s + 1) * TH, :] = res.results[core]["y"]
    return out


# revision 10
# speedup vs baseline: 1.0721x; 1.0001x over previous
"""Trainium2 Bass kernel for a dense transformer block (B=4, T=2048, C=1024, 16 heads).

Sharding over 8 NeuronCores: core i handles batch b=i//2 with shard s=i%2.
 - LN1 + QKV + causal attention for its 8 heads (c-slice [512s, 512s+512)) over full T
 - exchange of attention outputs within the (b) pair via 4 halved
   ReduceScatter ops (zero-region trick, fully SPMD-symmetric)
 - proj + LN2 + FFN + residuals on its t-half rows [1024s, 1024s+1024)

All GEMMs run in bf16 with fp32 PSUM accumulation. LayerNorm gain/bias are
folded into the weight matrices on the host; LN statistics use bn_stats and
rsqrt is sqrt(1/(var+eps)) with the reciprocal on the vector engine, so
the ACT table sets never thrash mid-phase.

Scheduling notes (engine queues are strict FIFO, so emission order matters):
 - PSUM uses two shared tags: "A" = 3x [128,1024] slots (GEMM-chain
   accumulators, QK scores) and "B" = 2x [128,512] slots (PE transposes, PV
   accumulators). Chains write both 512-wide halves of an "A" slot and are
   evacuated by a single wide ACT/DVE op.
 - q/k GEMM chains for head-pair hp+1 are emitted inside head-pair hp's
   attention loop, filling PE bubbles while the scalar engine runs exp().
 - The attention kk loop is software-pipelined (PV one step behind QK).
 - proj leaves the RS3 head-pair contribution to a second pass interleaved
   with LN2, so the PE never waits on the last ReduceScatter.
"""

from contextlib import ExitStack

import ml_dtypes
import numpy as np

import concourse.bass as bass
import concourse.mybir as mybir
import concourse.tile as tile
from concourse import bacc
from concourse.bass_utils import run_bass_kernel_spmd

f32 = mybir.dt.float32
bf16 = mybir.dt.bfloat16
AF = mybir.ActivationFunctionType
ALU = mybir.AluOpType
AX = mybir.AxisListType

B, T, C = 4, 2048, 1024
NH, D = 16, 64
F = 4 * C
H = C // 2            # per-core head c-slice (8 heads)
TH = T // 2           # per-core t-half for proj/FFN
EPS = 1e-5
RG = [[0, 1], [2, 3], [4, 5], [6, 7]]

_CACHE = {}


class S:
    """Shared build state."""
    pass


def _layernorm_tile(nc, st, xt, dst):
    """Row-standardize xt [128, C] -> dst [128, C] (bn_stats + ln/exp rsqrt)."""
    work = st.work
    st6 = work.tile([128, 2, 6], f32, name="st6", tag="st6")
    xg = xt[:].rearrange("p (g n) -> p g n", g=2)
    nc.vector.bn_stats(st6[:, 0, :], xg[:, 0, :])
    nc.vector.bn_stats(st6[:, 1, :], xg[:, 1, :])
    mv = work.tile([128, 2], f32, name="mv", tag="mv")
    nc.vector.bn_aggr(mv[:], st6[:])
    veps = work.tile([128, 1], f32, name="veps", tag="veps")
    nc.vector.tensor_scalar_add(veps[:], mv[:, 1:2], EPS)
    riv = work.tile([128, 1], f32, name="riv", tag="riv")
    with nc.allow_low_precision(reason="LN rsqrt"):
        nc.vector.reciprocal(riv[:], veps[:])
    rsig = work.tile([128, 1], f32, name="rsig", tag="rsig")
    nc.scalar.activation(rsig[:], riv[:], AF.Sqrt)
    nmu = work.tile([128, 1], f32, name="nmu", tag="nmu")
    nc.vector.tensor_tensor(nmu[:], mv[:, 0:1], rsig[:], ALU.mult)
    nc.vector.tensor_scalar_mul(nmu[:], nmu[:], -1.0)
    nc.scalar.activation(dst[:], xt[:], AF.Identity, bias=nmu[:], scale=rsig[:])


def _phase_hcol(nc, st):
    """LN1 + transpose (hcol = h^T for all 4 chunks) and the v GEMMs."""
    ps_t = st.ps_t
    st.qkvp = st.tc.tile_pool(name="qkv", bufs=1)
    qkv = st.qkvp.__enter__()
    st.wqkvp = st.tc.tile_pool(name="wqkv", bufs=1)
    wqkv = st.wqkvp.__enter__()
    st.htcp = st.tc.tile_pool(name="htc", bufs=1)
    htc = st.htcp.__enter__()
    st.xhp = st.tc.tile_pool(name="xh", bufs=4)
    xh = st.xhp.__enter__()

    st.wq_sb = [wqkv.tile([128, H], bf16, name=f"wq{k}", tag=f"wq{k}")
                for k in range(8)]
    st.wk_sb = [wqkv.tile([128, H], bf16, name=f"wk{k}", tag=f"wk{k}")
                for k in range(8)]
    wv_sb = [wqkv.tile([128, H], bf16, name=f"wv{k}", tag=f"wv{k}")
             for k in range(8)]
    for k in range(8):
        nc.gpsimd.dma_start(st.wq_sb[k][:], st.wq_h[k * 128:(k + 1) * 128, :])
        nc.gpsimd.dma_start(st.wk_sb[k][:], st.wk_h[k * 128:(k + 1) * 128, :])
        nc.gpsimd.dma_start(wv_sb[k][:], st.wv_h[k * 128:(k + 1) * 128, :])

    st.qT = [qkv.tile([128, T], bf16, name=f"qT{i}", tag=f"qT{i}") for i in range(4)]
    st.kT = [qkv.tile([128, T], bf16, name=f"kT{i}", tag=f"kT{i}") for i in range(4)]
    st.vn = [qkv.tile([128, 520], bf16, name=f"vn{i}", tag=f"vn{i}")
             for i in range(16)]
    st.hcol = [htc.tile([128, 8 * 512], bf16, name=f"hcol{j}", tag=f"hcol{j}")
               for j in range(4)]

    def emit_vchunk(j):
        # v GEMM for chunk j (natural layout, strided into vn + ones col)
        for tt4 in range(4):
            tt = j * 4 + tt4
            pg = ps_t("A")
            for k in range(8):
                nc.tensor.matmul(
                    pg[:],
                    st.hcol[j][:, k * 512 + tt4 * 128:k * 512 + (tt4 + 1) * 128],
                    wv_sb[k][:], start=(k == 0), stop=False)
            nc.tensor.matmul(pg[:], st.onesr[:, 0:128], st.bv_sb[:],
                             start=False, stop=True)
            nc.scalar.copy(
                st.vn[tt][:, 0:520].rearrange("p (h e) -> p h e", h=8)[:, :, 0:64],
                pg[:].rearrange("p (h d) -> p h d", h=8))
            nc.vector.memset(
                st.vn[tt][:, 0:520].rearrange("p (h e) -> p h e", h=8)[:, :, 64:65],
                1.0)

    for j in range(4):  # t-chunks of 512
        for tt4 in range(4):  # t-tiles of 128 within the chunk
            tt = j * 4 + tt4
            xt = xh.tile([128, C], f32, name="xt", tag="xt")
            nc.sync.dma_start(xt[:], st.x_h[tt * 128:(tt + 1) * 128, :])
            ht = xh.tile([128, C], bf16, name="ht", tag="ht")
            _layernorm_tile(nc, st, xt, ht)
            for cc in range(8):
                ptr = ps_t("B", (128, 128), bf16)
                nc.tensor.transpose(ptr[:], ht[:, cc * 128:(cc + 1) * 128],
                                    st.ident[:])
                dst = st.hcol[j][:, cc * 512 + tt4 * 128:
                                cc * 512 + (tt4 + 1) * 128]
                if cc < 3:
                    nc.scalar.copy(dst, ptr[:])
                else:
                    nc.vector.tensor_copy(out=dst, in_=ptr[:])
        if j >= 1:
            emit_vchunk(j - 1)
    emit_vchunk(3)
    st.xhp.__exit__(None, None, None)


def _emit_qk_half(nc, st, hp, j, half):
    """q (half=0) or k (half=1) GEMM chain for (head-pair hp, t-chunk j)."""
    dstT, wsb, brow = (((st.qT, st.wq_sb, st.bq_sb),
                        (st.kT, st.wk_sb, st.bk_sb))[half])
    sl = st.ps_t("A")
    for k in range(8):
        nc.tensor.matmul(sl, wsb[k][:, hp * 128:(hp + 1) * 128],
                         st.hcol[j][:, k * 512:(k + 1) * 512],
                         start=(k == 0), stop=False)
    nc.tensor.matmul(sl, brow[0:1, hp * 128:(hp + 1) * 128],
                     st.onesr[:, 0:512], start=False, stop=True)
    nc.vector.tensor_copy(out=dstT[hp][:, j * 512:(j + 1) * 512], in_=sl)


def _phase_attention(nc, st):
    """Causal attention per head-pair; ships results via ReduceScatter.

    The kk loop is software-pipelined: PV(kk) is emitted after QK(kk+1), and
    the QK score PSUM tag is multi-buffered, so the PE streams QK matmuls
    while the scalar engine exponentiates the previous chunk. q/k GEMMs for
    the NEXT head-pair are interleaved at chunk boundaries.
    """
    ps_t = st.ps_t
    st.wop = st.tc.tile_pool(name="wop", bufs=1, side="right")
    wop = st.wop.__enter__()
    st.attp = st.tc.tile_pool(name="attp", bufs=1, side="right")
    attp = st.attp.__enter__()
    st.attsbp = st.tc.tile_pool(name="attsb", bufs=1, side="right")
    attsb = st.attsbp.__enter__()

    attA = [attp.tile([128, T], bf16, name=f"attA{i}", tag=f"attA{i}")
            for i in range(4)]
    st.attA = attA
    st.att_sb = [attsb.tile([128, TH], bf16, name=f"asb{k}", tag=f"asb{k}")
                 for k in range(4)]
    aw = st.tc.tile_pool(name="aw", bufs=2)
    st.awp = aw
    aw = aw.__enter__()
    st.wo_sb = [wop.tile([128, C], bf16, name=f"wo{k}", tag=f"wo{k}")
                for k in range(8)]
    for k in range(8):
        nc.gpsimd.dma_start(st.wo_sb[k][:], st.wo_h[k * 128:(k + 1) * 128, :])

    def emit_pv(hp, kk, nk, r, ptb, po):
        for bi in range(2):
            h = 2 * hp + bi
            nc.tensor.matmul(
                po[bi][0:65, r:512],
                st.vn[kk][:, 65 * h:65 * h + 65],
                ptb[:, bi * 512 + r:bi * 512 + 512],
                start=(kk == 0), stop=(kk == nk - 1))

    def emit_norm(hp, j, po):
        tq0 = j * 512
        sj = j // 2
        for bi, b0 in enumerate((0, 64)):
            rs_row = aw.tile([1, 512], bf16, name="rs_row", tag="rsrow")
            nc.vector.tensor_copy(out=rs_row[:], in_=po[bi][64:65, :])
            pb = ps_t("A", (64, 512))
            nc.tensor.matmul(pb[:], st.onesr[:, 0:64], rs_row[:],
                             start=True, stop=True)
            rbi = aw.tile([64, 512], f32, name="rbi", tag="rbi")
            nc.vector.reciprocal_approx_fast(rbi[:], pb[:])
            attF = aw.tile([64, 512], bf16, name="attF", tag="attF")
            nc.vector.tensor_tensor(attF[:], po[bi][0:64, :], rbi[:],
                                    ALU.mult)
            nc.vector.tensor_scalar_mul(
                attA[hp][b0:b0 + 64, tq0:tq0 + 512], attF[:],
                st.sel_sb[0:64, sj:sj + 1])
            attBc = aw.tile([64, 512], bf16, name="attBc", tag="attBc")
            nc.vector.tensor_scalar_mul(attBc[:], attF[:],
                                        st.seln_sb[0:64, sj:sj + 1])
            nc.sync.dma_start(
                st.rs_in[hp][sj, b0:b0 + 64,
                             (j % 2) * 512:(j % 2) * 512 + 512],
                attBc[:])

    for j in range(4):
        _emit_qk_half(nc, st, 0, j, 0)
        _emit_qk_half(nc, st, 0, j, 1)
    for hp in range(4):
        norm_pending = None
        for j in range(4):
            tq0 = j * 512
            nk = 4 * (j + 1)
            po = [ps_t("pv"), ps_t("pv")]
            pending = None
            for kk in range(nk):
                r = 128 * (kk - 4 * j) if kk >= 4 * j else 0
                pqk = ps_t("A", (128, 1024))
                for bi, b0 in enumerate((0, 64)):
                    nc.tensor.matmul(
                        pqk[:, bi * 512 + r:bi * 512 + 512],
                        st.kT[hp][b0:b0 + 64, kk * 128:(kk + 1) * 128],
                        st.qT[hp][b0:b0 + 64, tq0 + r:tq0 + 512],
                        start=True, stop=True)
                ptb = st.ptp.tile([128, 1024], bf16, name="ptb", tag="pt")
                if r == 0:
                    nc.scalar.activation(ptb[:], pqk[:], AF.Exp)
                else:
                    nc.scalar.activation(
                        ptb[:].rearrange("p (b w) -> p b w", b=2)[:, :, r:512],
                        pqk[:].rearrange("p (b w) -> p b w", b=2)[:, :, r:512],
                        AF.Exp)
                if kk >= 4 * j:
                    nc.vector.tensor_tensor(
                        ptb[:].rearrange("p (b w) -> p b w", b=2)[:, :, r:r + 128],
                        ptb[:].rearrange("p (b w) -> p b w", b=2)[:, :, r:r + 128],
                        st.tri[:, None, :].to_broadcast((128, 2, 128)),
                        ALU.mult)
                if pending is not None:
                    emit_pv(hp, *pending)
                pending = (kk, nk, r, ptb, po)
                # All 8 q/k half-chains for the next head-pair are slotted
                # into the long (ACT-backlogged) chunks j=2 and j=3, so the
                # chunk boundaries never delay the QK->exp feed.
                if hp < 3:
                    sched = {(2, 2): (0, 0), (2, 5): (0, 1), (2, 8): (1, 0),
                             (3, 2): (1, 1), (3, 5): (2, 0), (3, 8): (2, 1),
                             (3, 11): (3, 0), (3, 14): (3, 1)}.get((j, kk))
                    if sched is not None:
                        _emit_qk_half(nc, st, hp + 1, sched[0], sched[1])
                if kk == 3 and norm_pending is not None:
                    emit_norm(hp, j - 1, norm_pending)
                    norm_pending = None
            emit_pv(hp, *pending)
            norm_pending = po
        emit_norm(hp, 3, norm_pending)

        nc.gpsimd.collective_compute(
            "ReduceScatter", ALU.add, replica_groups=RG,
            ins=[st.rs_in[hp][:]], outs=[st.rs_out[hp][:]])
        nc.gpsimd.dma_start(st.att_sb[hp][:], st.rs_out[hp][:])


def _phase_proj(nc, st):
    """Projection pass A: local heads + first three exchanged head-pairs +
    residual. The RS3 contribution is added during LN2 (pass B)."""
    ps_t = st.ps_t
    st.awp.__exit__(None, None, None)
    st.htcp.__exit__(None, None, None)
    st.wqkvp.__exit__(None, None, None)
    st.qkvp.__exit__(None, None, None)
    st.x2p = st.tc.tile_pool(name="x2p", bufs=1)
    x2p = st.x2p.__enter__()
    st.latebp = st.tc.tile_pool(name="lateb", bufs=1)
    lateb = st.latebp.__enter__()
    st.xrpp = st.tc.tile_pool(name="xrp", bufs=2)
    xrp = st.xrpp.__enter__()

    st.b2_sb = lateb.tile([1, C], bf16, name="b2_sb")
    nc.sync.dma_start(st.b2_sb[:], st.b2_h[:])
    st.x2 = [x2p.tile([128, C], f32, name=f"x2_{t}", tag=f"x2_{t}")
             for t in range(8)]
    for tt in range(8):
        xr = xrp.tile([128, C], f32, name="xr", tag="xr")
        nc.sync.dma_start(xr[:], st.xres_h[tt * 128:(tt + 1) * 128, :])
        pg = ps_t("A", (128, 1024))
        for cc in range(2):
            sl = pg[:, cc * 512:(cc + 1) * 512]
            for k in range(4):
                for half in range(2):
                    nc.tensor.matmul(
                        sl,
                        st.attA[k][:, half * TH + tt * 128:
                                   half * TH + (tt + 1) * 128],
                        st.wo_sb[k][:, cc * 512:(cc + 1) * 512],
                        start=(k == 0 and half == 0), stop=False)
            for k in range(3):
                nc.tensor.matmul(sl, st.att_sb[k][:, tt * 128:(tt + 1) * 128],
                                 st.wo_sb[4 + k][:, cc * 512:(cc + 1) * 512],
                                 start=False, stop=(k == 2))
        nc.vector.tensor_tensor(st.x2[tt][:], pg[:], xr[:], ALU.add)
    st.xrpp.__exit__(None, None, None)


def _phase_ln2(nc, st):
    """proj pass B (RS3 head-pair) + LN2 + transpose to h2T, interleaved."""
    ps_t = st.ps_t
    st.h2p = st.tc.tile_pool(name="h2p", bufs=1)
    h2p = st.h2p.__enter__()
    st.h2wp = st.tc.tile_pool(name="h2w", bufs=3)
    h2w = st.h2wp.__enter__()
    st.h2T = [h2p.tile([128, TH], bf16, name=f"h2T{k}", tag=f"h2T{k}")
              for k in range(8)]
    for tt in range(8):
        pgb = ps_t("A", (128, 1024))
        for cc in range(2):
            nc.tensor.matmul(pgb[:, cc * 512:(cc + 1) * 512],
                             st.att_sb[3][:, tt * 128:(tt + 1) * 128],
                             st.wo_sb[7][:, cc * 512:(cc + 1) * 512],
                             start=True, stop=True)
        nc.vector.tensor_tensor(st.x2[tt][:], pgb[:], st.x2[tt][:], ALU.add)
        h2t = h2w.tile([128, C], bf16, name="h2t", tag="h2t")
        _layernorm_tile(nc, st, st.x2[tt], h2t)
        for cc in range(8):
            ptr = ps_t("B", (128, 128), bf16)
            nc.tensor.transpose(ptr[:], h2t[:, cc * 128:(cc + 1) * 128],
                                st.ident[:])
            if cc < 5:
                nc.scalar.copy(st.h2T[cc][:, tt * 128:(tt + 1) * 128], ptr[:])
            else:
                nc.vector.tensor_copy(
                    out=st.h2T[cc][:, tt * 128:(tt + 1) * 128], in_=ptr[:])
    st.h2wp.__exit__(None, None, None)
    st.attsbp.__exit__(None, None, None)
    st.attp.__exit__(None, None, None)
    st.wop.__exit__(None, None, None)


def _phase_ffn(nc, st):
    """FFN with grouped ff-dim accumulation, residual, output DMA."""
    ps_t = st.ps_t
    yacp = st.tc.tile_pool(name="yac", bufs=1)
    yac = yacp.__enter__()
    w1pp = st.tc.tile_pool(name="w1p", bufs=16)
    w1p = w1pp.__enter__()
    w2pp = st.tc.tile_pool(name="w2p", bufs=8)
    w2p = w2pp.__enter__()
    utpp = st.tc.tile_pool(name="utp", bufs=8)
    utp = utpp.__enter__()

    y_acc = [yac.tile([128, C], f32, name=f"ya{t}", tag=f"ya{t}")
             for t in range(8)]
    for g in range(4):
        w1g = []
        for k in range(8):
            w1k = w1p.tile([128, 1024], bf16, name="w1k", tag="w1k")
            nc.sync.dma_start(w1k[:],
                              st.w1_h[k * 128:(k + 1) * 128,
                                      g * 1024:(g + 1) * 1024])
            w1g.append(w1k)
        ut_g = []
        for ff in range(8):
            f = g * 8 + ff
            ut = utp.tile([128, TH], bf16, name="ut", tag="ut")
            pg = ps_t("A", (128, 1024))
            for tch in range(2):
                sl = pg[:, tch * 512:(tch + 1) * 512]
                for k in range(8):
                    nc.tensor.matmul(sl, w1g[k][:, ff * 128:(ff + 1) * 128],
                                     st.h2T[k][:, tch * 512:(tch + 1) * 512],
                                     start=(k == 0), stop=(k == 7))
            nc.scalar.activation(ut[:], pg[:], AF.Relu,
                                 bias=st.b1_sb[:, f:f + 1])
            ut_g.append(ut)
        w2g = []
        for ff in range(8):
            f = g * 8 + ff
            w2t = w2p.tile([128, C], bf16, name="w2t", tag="w2t")
            nc.sync.dma_start(w2t[:], st.w2_h[f * 128:(f + 1) * 128, :])
            w2g.append(w2t)
        for tt in range(8):
            pg = ps_t("A", (128, 1024))
            for cc in range(2):
                sl = pg[:, cc * 512:(cc + 1) * 512]
                for ff in range(8):
                    nc.tensor.matmul(sl, ut_g[ff][:, tt * 128:(tt + 1) * 128],
                                     w2g[ff][:, cc * 512:(cc + 1) * 512],
                                     start=(ff == 0),
                                     stop=(False if g == 0 else ff == 7))
                if g == 0:
                    nc.tensor.matmul(sl, st.onesr[:, 0:128],
                                     st.b2_sb[:, cc * 512:(cc + 1) * 512],
                                     start=False, stop=True)
            if g == 0:
                nc.vector.tensor_tensor(y_acc[tt][:], pg[:], st.x2[tt][:],
                                        ALU.add)
            else:
                nc.vector.tensor_tensor(y_acc[tt][:], pg[:], y_acc[tt][:],
                                        ALU.add)
    for tt in range(8):
        nc.sync.dma_start(st.y_h[tt * 128:(tt + 1) * 128, :], y_acc[tt][:])
    utpp.__exit__(None, None, None)
    w2pp.__exit__(None, None, None)
    w1pp.__exit__(None, None, None)
    yacp.__exit__(None, None, None)
    st.h2p.__exit__(None, None, None)
    st.latebp.__exit__(None, None, None)
    st.x2p.__exit__(None, None, None)


def build_program():
    if "nc" in _CACHE:
        return _CACHE["nc"]
    nc = bacc.Bacc(None)
    st = S()

    st.x_h = nc.declare_dram_parameter("x", [T, C], f32, isOutput=False)
    st.xres_h = nc.declare_dram_parameter("xres", [TH, C], f32, isOutput=False)
    st.wq_h = nc.declare_dram_parameter("wq", [C, H], bf16, isOutput=False)
    st.wk_h = nc.declare_dram_parameter("wk", [C, H], bf16, isOutput=False)
    st.wv_h = nc.declare_dram_parameter("wv", [C, H], bf16, isOutput=False)
    bq_h = nc.declare_dram_parameter("bq", [1, H], bf16, isOutput=False)
    bk_h = nc.declare_dram_parameter("bk", [1, H], bf16, isOutput=False)
    bv_h = nc.declare_dram_parameter("bv", [1, H], bf16, isOutput=False)
    st.wo_h = nc.declare_dram_parameter("wo", [C, C], bf16, isOutput=False)
    st.w1_h = nc.declare_dram_parameter("w1", [C, F], bf16, isOutput=False)
    b1_h = nc.declare_dram_parameter("b1", [128, 32], f32, isOutput=False)
    st.w2_h = nc.declare_dram_parameter("w2", [F, C], bf16, isOutput=False)
    b2_h = nc.declare_dram_parameter("b2", [1, C], bf16, isOutput=False)
    ident_h = nc.declare_dram_parameter("ident", [128, 128], bf16, isOutput=False)
    tri_h = nc.declare_dram_parameter("tri", [128, 128], bf16, isOutput=False)
    onesr_h = nc.declare_dram_parameter("onesr", [1, 512], bf16, isOutput=False)
    sel_h = nc.declare_dram_parameter("sel", [128, 2], f32, isOutput=False)
    seln_h = nc.declare_dram_parameter("seln", [128, 2], f32, isOutput=False)
    st.y_h = nc.declare_dram_parameter("y", [TH, C], f32, isOutput=True)

    st.rs_in = [nc.dram_tensor(f"rs_in{hp}", [2, 128, TH], bf16)
                for hp in range(4)]
    st.rs_out = [nc.dram_tensor(f"rs_out{hp}", [128, TH], bf16)
                 for hp in range(4)]

    with tile.TileContext(nc) as tc, ExitStack() as stack:
        st.tc, st.stack = tc, stack
        cst = stack.enter_context(tc.tile_pool(name="const", bufs=1))
        ps = stack.enter_context(tc.tile_pool(name="ps", bufs=1, space="PSUM"))
        st.work = stack.enter_context(tc.tile_pool(name="work", bufs=4))
        st.ptp = stack.enter_context(tc.tile_pool(name="ptp", bufs=3))

        st.ident = cst.tile([128, 128], bf16, name="ident")
        st.tri = cst.tile([128, 128], bf16, name="tri")
        st.onesr = cst.tile([1, 512], bf16, name="onesr")
        st.bq_sb = cst.tile([1, H], bf16, name="bq_sb")
        st.bk_sb = cst.tile([1, H], bf16, name="bk_sb")
        st.bv_sb = cst.tile([1, H], bf16, name="bv_sb")
        st.b1_sb = cst.tile([128, 32], f32, name="b1_sb")
        st.sel_sb = cst.tile([128, 2], f32, name="sel_sb")
        st.seln_sb = cst.tile([128, 2], f32, name="seln_sb")
        for t_, h_ in [(st.ident, ident_h), (st.tri, tri_h), (st.onesr, onesr_h),
                       (st.bq_sb, bq_h), (st.bk_sb, bk_h),
                       (st.bv_sb, bv_h), (st.b1_sb, b1_h),
                       (st.sel_sb, sel_h), (st.seln_sb, seln_h)]:
            nc.sync.dma_start(t_[:], h_[:])
        st.b2_h = b2_h

        def ps_t(tag, shape=(128, 512), dt=f32):
            if tag == "A":
                assert shape[0] <= 128 and shape[1] <= 1024
                full = ps.tile([128, 1024], dt, tag="A", name="ps_A", bufs=2)
                return full[0:shape[0], 0:shape[1]]
            tag = "B" if tag == "pv" else tag
            return ps.tile(list(shape), dt, tag="B", name="ps_B", bufs=4)
        st.ps_t = ps_t

        _phase_hcol(nc, st)
        _phase_attention(nc, st)
        _phase_proj(nc, st)
        _phase_ln2(nc, st)
        _phase_ffn(nc, st)

    nc.compile()
    _CACHE["nc"] = nc
    return nc


def make_inputs(x, Wq, Wk, Wv, Wo, bo, W1, b1, W2, b2,
                ln1_g, ln1_b, ln2_g, ln2_b):
    """Build per-core input maps (host-side sharding + LN folding)."""
    x = np.asarray(x, np.float32)
    scale = float(C) ** -0.5

    wq_eff = ln1_g[:, None] * Wq
    wk_eff = ln1_g[:, None] * Wk * scale
    wv_eff = ln1_g[:, None] * Wv
    bq_full = ln1_b @ Wq
    bk_full = (ln1_b @ Wk) * scale
    bv_full = ln1_b @ Wv
    w1_eff = ln2_g[:, None] * W1
    b1_eff = b1 + ln2_b @ W1

    BF = ml_dtypes.bfloat16
    ident = np.eye(128, dtype=BF)
    tri = np.triu(np.ones((128, 128), BF))
    onesr = np.ones((1, 512), BF)

    in_maps = []
    for core in range(8):
        b, s = core // 2, core % 2
        cs = slice(s * H, (s + 1) * H)
        ts = slice(s * TH, (s + 1) * TH)
        own = np.arange(s * H, (s + 1) * H)
        other = np.arange((1 - s) * H, (2 - s) * H)
        perm = np.concatenate([own, other])
        in_maps.append({
            "x": np.ascontiguousarray(x[b]),
            "xres": np.ascontiguousarray(x[b, ts, :] + bo[None, :]),
            "wq": np.ascontiguousarray(wq_eff[:, cs].astype(BF)),
            "wk": np.ascontiguousarray(wk_eff[:, cs].astype(BF)),
            "wv": np.ascontiguousarray(wv_eff[:, cs].astype(BF)),
            "bq": np.ascontiguousarray(bq_full[cs].reshape(1, H).astype(BF)),
            "bk": np.ascontiguousarray(bk_full[cs].reshape(1, H).astype(BF)),
            "bv": np.ascontiguousarray(bv_full[cs].reshape(1, H).astype(BF)),
            "wo": np.ascontiguousarray(Wo[perm, :].astype(BF)),
            "w1": np.ascontiguousarray(w1_eff.astype(BF)),
            "b1": np.ascontiguousarray(b1_eff.reshape(32, 128).T),
            "w2": np.ascontiguousarray(W2.astype(BF)),
            "b2": np.ascontiguousarray(b2.reshape(1, C).astype(BF)),
            "ident": ident, "tri": tri, "onesr": onesr,
            "sel": np.tile(np.eye(2, dtype=np.float32)[s][None, :], (128, 1)),
            "seln": np.tile(np.eye(2, dtype=np.float32)[1 - s][None, :], (128, 1)),
        })
    return in_maps


def kernel(**inputs):
    nc = build_program()
    in_maps = make_inputs(**{k: np.asarray(v, np.float32) for k, v in inputs.items()})
    res = run_bass_kernel_spmd(nc, in_maps, list(range(8)))
    out = np.empty((B, T, C), np.float32)
    for core in range(8):
        b, s = core // 2, core % 2
        out[b, s * TH:(s + 1) * TH, :] = res.results[core]["y"]
    return out


# revision 11
# speedup vs baseline: 1.0908x; 1.0174x over previous
"""Trainium2 Bass kernel for a dense transformer block (B=4, T=2048, C=1024, 16 heads).

Sharding over 8 NeuronCores: core i handles batch b=i//2 with shard s=i%2.
 - LN1 + QKV + causal attention for its 8 heads (c-slice [512s, 512s+512)) over full T
 - exchange of attention outputs within the (b) pair via 4 halved
   ReduceScatter ops (zero-region trick, fully SPMD-symmetric)
 - proj + LN2 + FFN + residuals on its t-half rows [1024s, 1024s+1024)

All GEMMs run in bf16 with fp32 PSUM accumulation. LayerNorm gain/bias are
folded into the weight matrices on the host; LN statistics use bn_stats and
rsqrt is sqrt(1/(var+eps)) with the reciprocal on the vector engine, so
the ACT table sets never thrash mid-phase.

Scheduling notes (engine queues are strict FIFO, so emission order matters):
 - PSUM uses two shared tags: "A" = 3x [128,1024] slots (GEMM-chain
   accumulators, QK scores) and "B" = 2x [128,512] slots (PE transposes, PV
   accumulators). Chains write both 512-wide halves of an "A" slot and are
   evacuated by a single wide ACT/DVE op.
 - q/k GEMM chains for head-pair hp+1 are emitted inside head-pair hp's
   attention loop, filling PE bubbles while the scalar engine runs exp().
 - The attention kk loop is software-pipelined with PV two steps behind
   QK, so PV never waits on exp() and the loop runs at the exp rate.
 - proj leaves the RS3 head-pair contribution to a second pass interleaved
   with LN2, so the PE never waits on the last ReduceScatter.
"""

from contextlib import ExitStack

import ml_dtypes
import numpy as np

import concourse.bass as bass
import concourse.mybir as mybir
import concourse.tile as tile
from concourse import bacc
from concourse.bass_utils import run_bass_kernel_spmd

f32 = mybir.dt.float32
bf16 = mybir.dt.bfloat16
AF = mybir.ActivationFunctionType
ALU = mybir.AluOpType
AX = mybir.AxisListType

B, T, C = 4, 2048, 1024
NH, D = 16, 64
F = 4 * C
H = C // 2            # per-core head c-slice (8 heads)
TH = T // 2           # per-core t-half for proj/FFN
EPS = 1e-5
RG = [[0, 1], [2, 3], [4, 5], [6, 7]]

_CACHE = {}


class S:
    """Shared build state."""
    pass


def _layernorm_tile(nc, st, xt, dst):
    """Row-standardize xt [128, C] -> dst [128, C] (bn_stats + ln/exp rsqrt)."""
    work = st.work
    st6 = work.tile([128, 2, 6], f32, name="st6", tag="st6")
    xg = xt[:].rearrange("p (g n) -> p g n", g=2)
    nc.vector.bn_stats(st6[:, 0, :], xg[:, 0, :])
    nc.vector.bn_stats(st6[:, 1, :], xg[:, 1, :])
    mv = work.tile([128, 2], f32, name="mv", tag="mv")
    nc.vector.bn_aggr(mv[:], st6[:])
    veps = work.tile([128, 1], f32, name="veps", tag="veps")
    nc.vector.tensor_scalar_add(veps[:], mv[:, 1:2], EPS)
    riv = work.tile([128, 1], f32, name="riv", tag="riv")
    with nc.allow_low_precision(reason="LN rsqrt"):
        nc.vector.reciprocal(riv[:], veps[:])
    rsig = work.tile([128, 1], f32, name="rsig", tag="rsig")
    nc.scalar.activation(rsig[:], riv[:], AF.Sqrt)
    nmu = work.tile([128, 1], f32, name="nmu", tag="nmu")
    nc.vector.tensor_tensor(nmu[:], mv[:, 0:1], rsig[:], ALU.mult)
    nc.vector.tensor_scalar_mul(nmu[:], nmu[:], -1.0)
    nc.scalar.activation(dst[:], xt[:], AF.Identity, bias=nmu[:], scale=rsig[:])


def _phase_hcol(nc, st):
    """LN1 + transpose (hcol = h^T for all 4 chunks) and the v GEMMs."""
    ps_t = st.ps_t
    st.qkvp = st.tc.tile_pool(name="qkv", bufs=1)
    qkv = st.qkvp.__enter__()
    st.wqkvp = st.tc.tile_pool(name="wqkv", bufs=1)
    wqkv = st.wqkvp.__enter__()
    st.htcp = st.tc.tile_pool(name="htc", bufs=1)
    htc = st.htcp.__enter__()
    st.xhp = st.tc.tile_pool(name="xh", bufs=4)
    xh = st.xhp.__enter__()

    st.wq_sb = [wqkv.tile([128, H], bf16, name=f"wq{k}", tag=f"wq{k}")
                for k in range(8)]
    st.wk_sb = [wqkv.tile([128, H], bf16, name=f"wk{k}", tag=f"wk{k}")
                for k in range(8)]
    wv_sb = [wqkv.tile([128, H], bf16, name=f"wv{k}", tag=f"wv{k}")
             for k in range(8)]
    for k in range(8):
        nc.gpsimd.dma_start(st.wq_sb[k][:], st.wq_h[k * 128:(k + 1) * 128, :])
        nc.gpsimd.dma_start(st.wk_sb[k][:], st.wk_h[k * 128:(k + 1) * 128, :])
        nc.gpsimd.dma_start(wv_sb[k][:], st.wv_h[k * 128:(k + 1) * 128, :])

    st.qT = [qkv.tile([128, T], bf16, name=f"qT{i}", tag=f"qT{i}") for i in range(4)]
    st.kT = [qkv.tile([128, T], bf16, name=f"kT{i}", tag=f"kT{i}") for i in range(4)]
    st.vn = [qkv.tile([128, 520], bf16, name=f"vn{i}", tag=f"vn{i}")
             for i in range(16)]
    st.hcol = [htc.tile([128, 8 * 512], bf16, name=f"hcol{j}", tag=f"hcol{j}")
               for j in range(4)]

    def emit_vchunk(j):
        # v GEMM for chunk j (natural layout, strided into vn + ones col)
        for tt4 in range(4):
            tt = j * 4 + tt4
            pg = ps_t("A")
            for k in range(8):
                nc.tensor.matmul(
                    pg[:],
                    st.hcol[j][:, k * 512 + tt4 * 128:k * 512 + (tt4 + 1) * 128],
                    wv_sb[k][:], start=(k == 0), stop=False)
            nc.tensor.matmul(pg[:], st.onesr[:, 0:128], st.bv_sb[:],
                             start=False, stop=True)
            nc.scalar.copy(
                st.vn[tt][:, 0:520].rearrange("p (h e) -> p h e", h=8)[:, :, 0:64],
                pg[:].rearrange("p (h d) -> p h d", h=8))
            nc.vector.memset(
                st.vn[tt][:, 0:520].rearrange("p (h e) -> p h e", h=8)[:, :, 64:65],
                1.0)

    for j in range(4):  # t-chunks of 512
        for tt4 in range(4):  # t-tiles of 128 within the chunk
            tt = j * 4 + tt4
            xt = xh.tile([128, C], f32, name="xt", tag="xt")
            nc.sync.dma_start(xt[:], st.x_h[tt * 128:(tt + 1) * 128, :])
            ht = xh.tile([128, C], bf16, name="ht", tag="ht")
            _layernorm_tile(nc, st, xt, ht)
            for cc in range(8):
                ptr = ps_t("B", (128, 128), bf16)
                nc.tensor.transpose(ptr[:], ht[:, cc * 128:(cc + 1) * 128],
                                    st.ident[:])
                dst = st.hcol[j][:, cc * 512 + tt4 * 128:
                                cc * 512 + (tt4 + 1) * 128]
                if cc < 3:
                    nc.scalar.copy(dst, ptr[:])
                else:
                    nc.vector.tensor_copy(out=dst, in_=ptr[:])
        if j >= 1:
            emit_vchunk(j - 1)
    emit_vchunk(3)
    st.xhp.__exit__(None, None, None)


def _emit_qk_half(nc, st, hp, j, half):
    """q (half=0) or k (half=1) GEMM chain for (head-pair hp, t-chunk j)."""
    dstT, wsb, brow = (((st.qT, st.wq_sb, st.bq_sb),
                        (st.kT, st.wk_sb, st.bk_sb))[half])
    sl = st.ps_t("A")
    for k in range(8):
        nc.tensor.matmul(sl, wsb[k][:, hp * 128:(hp + 1) * 128],
                         st.hcol[j][:, k * 512:(k + 1) * 512],
                         start=(k == 0), stop=False)
    nc.tensor.matmul(sl, brow[0:1, hp * 128:(hp + 1) * 128],
                     st.onesr[:, 0:512], start=False, stop=True)
    nc.vector.tensor_copy(out=dstT[hp][:, j * 512:(j + 1) * 512], in_=sl)


def _phase_attention(nc, st):
    """Causal attention per head-pair; ships results via ReduceScatter.

    The kk loop is software-pipelined: PV(kk) is emitted after QK(kk+1), and
    the QK score PSUM tag is multi-buffered, so the PE streams QK matmuls
    while the scalar engine exponentiates the previous chunk. q/k GEMMs for
    the NEXT head-pair are interleaved at chunk boundaries.
    """
    ps_t = st.ps_t
    st.wop = st.tc.tile_pool(name="wop", bufs=1, side="right")
    wop = st.wop.__enter__()
    st.attp = st.tc.tile_pool(name="attp", bufs=1, side="right")
    attp = st.attp.__enter__()
    st.attsbp = st.tc.tile_pool(name="attsb", bufs=1, side="right")
    attsb = st.attsbp.__enter__()

    attA = [attp.tile([128, T], bf16, name=f"attA{i}", tag=f"attA{i}")
            for i in range(4)]
    st.attA = attA
    st.att_sb = [attsb.tile([128, TH], bf16, name=f"asb{k}", tag=f"asb{k}")
                 for k in range(4)]
    aw = st.tc.tile_pool(name="aw", bufs=2)
    st.awp = aw
    aw = aw.__enter__()
    st.wo_sb = [wop.tile([128, C], bf16, name=f"wo{k}", tag=f"wo{k}")
                for k in range(8)]
    for k in range(8):
        nc.gpsimd.dma_start(st.wo_sb[k][:], st.wo_h[k * 128:(k + 1) * 128, :])

    def emit_pv(hp, kk, nk, r, ptb, po):
        for bi in range(2):
            h = 2 * hp + bi
            nc.tensor.matmul(
                po[bi][0:65, r:512],
                st.vn[kk][:, 65 * h:65 * h + 65],
                ptb[:, bi * 512 + r:bi * 512 + 512],
                start=(kk == 0), stop=(kk == nk - 1))

    def emit_norm(hp, j, po):
        tq0 = j * 512
        sj = j // 2
        for bi, b0 in enumerate((0, 64)):
            rs_row = aw.tile([1, 512], bf16, name="rs_row", tag="rsrow")
            nc.vector.tensor_copy(out=rs_row[:], in_=po[bi][64:65, :])
            pb = ps_t("A", (64, 512))
            nc.tensor.matmul(pb[:], st.onesr[:, 0:64], rs_row[:],
                             start=True, stop=True)
            rbi = aw.tile([64, 512], f32, name="rbi", tag="rbi")
            nc.vector.reciprocal_approx_fast(rbi[:], pb[:])
            attF = aw.tile([64, 512], bf16, name="attF", tag="attF")
            nc.vector.tensor_tensor(attF[:], po[bi][0:64, :], rbi[:],
                                    ALU.mult)
            nc.vector.tensor_scalar_mul(
                attA[hp][b0:b0 + 64, tq0:tq0 + 512], attF[:],
                st.sel_sb[0:64, sj:sj + 1])
            attBc = aw.tile([64, 512], bf16, name="attBc", tag="attBc")
            nc.vector.tensor_scalar_mul(attBc[:], attF[:],
                                        st.seln_sb[0:64, sj:sj + 1])
            nc.sync.dma_start(
                st.rs_in[hp][sj, b0:b0 + 64,
                             (j % 2) * 512:(j % 2) * 512 + 512],
                attBc[:])

    for j in range(4):
        _emit_qk_half(nc, st, 0, j, 0)
        _emit_qk_half(nc, st, 0, j, 1)
    for hp in range(4):
        norm_pending = None
        for j in range(4):
            tq0 = j * 512
            nk = 4 * (j + 1)
            po = [ps_t("pv"), ps_t("pv")]
            pending = []
            for kk in range(nk):
                r = 128 * (kk - 4 * j) if kk >= 4 * j else 0
                pqk = ps_t("A", (128, 1024))
                for bi, b0 in enumerate((0, 64)):
                    nc.tensor.matmul(
                        pqk[:, bi * 512 + r:bi * 512 + 512],
                        st.kT[hp][b0:b0 + 64, kk * 128:(kk + 1) * 128],
                        st.qT[hp][b0:b0 + 64, tq0 + r:tq0 + 512],
                        start=True, stop=True)
                ptb = st.ptp.tile([128, 1024], bf16, name="ptb", tag="pt")
                if r == 0:
                    nc.scalar.activation(ptb[:], pqk[:], AF.Exp)
                else:
                    nc.scalar.activation(
                        ptb[:].rearrange("p (b w) -> p b w", b=2)[:, :, r:512],
                        pqk[:].rearrange("p (b w) -> p b w", b=2)[:, :, r:512],
                        AF.Exp)
                if kk >= 4 * j:
                    nc.vector.tensor_tensor(
                        ptb[:].rearrange("p (b w) -> p b w", b=2)[:, :, r:r + 128],
                        ptb[:].rearrange("p (b w) -> p b w", b=2)[:, :, r:r + 128],
                        st.tri[:, None, :].to_broadcast((128, 2, 128)),
                        ALU.mult)
                if len(pending) == 2:
                    emit_pv(hp, *pending.pop(0))
                pending.append((kk, nk, r, ptb, po))
                # All 8 q/k half-chains for the next head-pair are slotted
                # into the long (ACT-backlogged) chunks j=2 and j=3, so the
                # chunk boundaries never delay the QK->exp feed.
                if hp < 3:
                    sched = {(2, 2): (0, 0), (2, 5): (0, 1), (2, 8): (1, 0),
                             (3, 2): (1, 1), (3, 5): (2, 0), (3, 8): (2, 1),
                             (3, 11): (3, 0), (3, 14): (3, 1)}.get((j, kk))
                    if sched is not None:
                        _emit_qk_half(nc, st, hp + 1, sched[0], sched[1])
                if kk == 3 and norm_pending is not None:
                    emit_norm(hp, j - 1, norm_pending)
                    norm_pending = None
            for p in pending:
                emit_pv(hp, *p)
            norm_pending = po
        emit_norm(hp, 3, norm_pending)

        nc.gpsimd.collective_compute(
            "ReduceScatter", ALU.add, replica_groups=RG,
            ins=[st.rs_in[hp][:]], outs=[st.rs_out[hp][:]])
        nc.gpsimd.dma_start(st.att_sb[hp][:], st.rs_out[hp][:])


def _phase_proj(nc, st):
    """Projection pass A: local heads + first three exchanged head-pairs +
    residual. The RS3 contribution is added during LN2 (pass B)."""
    ps_t = st.ps_t
    st.awp.__exit__(None, None, None)
    st.htcp.__exit__(None, None, None)
    st.wqkvp.__exit__(None, None, None)
    st.qkvp.__exit__(None, None, None)
    st.x2p = st.tc.tile_pool(name="x2p", bufs=1)
    x2p = st.x2p.__enter__()
    st.latebp = st.tc.tile_pool(name="lateb", bufs=1)
    lateb = st.latebp.__enter__()
    st.xrpp = st.tc.tile_pool(name="xrp", bufs=2)
    xrp = st.xrpp.__enter__()

    st.b2_sb = lateb.tile([1, C], bf16, name="b2_sb")
    nc.sync.dma_start(st.b2_sb[:], st.b2_h[:])
    st.x2 = [x2p.tile([128, C], f32, name=f"x2_{t}", tag=f"x2_{t}")
             for t in range(8)]
    for tt in range(8):
        xr = xrp.tile([128, C], f32, name="xr", tag="xr")
        nc.sync.dma_start(xr[:], st.xres_h[tt * 128:(tt + 1) * 128, :])
        pg = ps_t("A", (128, 1024))
        for cc in range(2):
            sl = pg[:, cc * 512:(cc + 1) * 512]
            for k in range(4):
                for half in range(2):
                    nc.tensor.matmul(
                        sl,
                        st.attA[k][:, half * TH + tt * 128:
                                   half * TH + (tt + 1) * 128],
                        st.wo_sb[k][:, cc * 512:(cc + 1) * 512],
                        start=(k == 0 and half == 0), stop=False)
            for k in range(3):
                nc.tensor.matmul(sl, st.att_sb[k][:, tt * 128:(tt + 1) * 128],
                                 st.wo_sb[4 + k][:, cc * 512:(cc + 1) * 512],
                                 start=False, stop=(k == 2))
        nc.vector.tensor_tensor(st.x2[tt][:], pg[:], xr[:], ALU.add)
    st.xrpp.__exit__(None, None, None)


def _phase_ln2(nc, st):
    """proj pass B (RS3 head-pair) + LN2 + transpose to h2T, interleaved."""
    ps_t = st.ps_t
    st.h2p = st.tc.tile_pool(name="h2p", bufs=1)
    h2p = st.h2p.__enter__()
    st.h2wp = st.tc.tile_pool(name="h2w", bufs=3)
    h2w = st.h2wp.__enter__()
    st.h2T = [h2p.tile([128, TH], bf16, name=f"h2T{k}", tag=f"h2T{k}")
              for k in range(8)]
    for tt in range(8):
        pgb = ps_t("A", (128, 1024))
        for cc in range(2):
            nc.tensor.matmul(pgb[:, cc * 512:(cc + 1) * 512],
                             st.att_sb[3][:, tt * 128:(tt + 1) * 128],
                             st.wo_sb[7][:, cc * 512:(cc + 1) * 512],
                             start=True, stop=True)
        nc.vector.tensor_tensor(st.x2[tt][:], pgb[:], st.x2[tt][:], ALU.add)
        h2t = h2w.tile([128, C], bf16, name="h2t", tag="h2t")
        _layernorm_tile(nc, st, st.x2[tt], h2t)
        for cc in range(8):
            ptr = ps_t("B", (128, 128), bf16)
            nc.tensor.transpose(ptr[:], h2t[:, cc * 128:(cc + 1) * 128],
                                st.ident[:])
            if cc < 5:
                nc.scalar.copy(st.h2T[cc][:, tt * 128:(tt + 1) * 128], ptr[:])
            else:
                nc.vector.tensor_copy(
                    out=st.h2T[cc][:, tt * 128:(tt + 1) * 128], in_=ptr[:])
    st.h2wp.__exit__(None, None, None)
    st.attsbp.__exit__(None, None, None)
    st.attp.__exit__(None, None, None)
    st.wop.__exit__(None, None, None)


def _phase_ffn(nc, st):
    """FFN with grouped ff-dim accumulation, residual, output DMA."""
    ps_t = st.ps_t
    yacp = st.tc.tile_pool(name="yac", bufs=1)
    yac = yacp.__enter__()
    w1pp = st.tc.tile_pool(name="w1p", bufs=16)
    w1p = w1pp.__enter__()
    w2pp = st.tc.tile_pool(name="w2p", bufs=8)
    w2p = w2pp.__enter__()
    utpp = st.tc.tile_pool(name="utp", bufs=8)
    utp = utpp.__enter__()

    y_acc = [yac.tile([128, C], f32, name=f"ya{t}", tag=f"ya{t}")
             for t in range(8)]
    for g in range(4):
        w1g = []
        for k in range(8):
            w1k = w1p.tile([128, 1024], bf16, name="w1k", tag="w1k")
            nc.sync.dma_start(w1k[:],
                              st.w1_h[k * 128:(k + 1) * 128,
                                      g * 1024:(g + 1) * 1024])
            w1g.append(w1k)
        ut_g = []
        for ff in range(8):
            f = g * 8 + ff
            ut = utp.tile([128, TH], bf16, name="ut", tag="ut")
            pg = ps_t("A", (128, 1024))
            for tch in range(2):
                sl = pg[:, tch * 512:(tch + 1) * 512]
                for k in range(8):
                    nc.tensor.matmul(sl, w1g[k][:, ff * 128:(ff + 1) * 128],
                                     st.h2T[k][:, tch * 512:(tch + 1) * 512],
                                     start=(k == 0), stop=(k == 7))
            nc.scalar.activation(ut[:], pg[:], AF.Relu,
                                 bias=st.b1_sb[:, f:f + 1])
            ut_g.append(ut)
        w2g = []
        for ff in range(8):
            f = g * 8 + ff
            w2t = w2p.tile([128, C], bf16, name="w2t", tag="w2t")
            nc.sync.dma_start(w2t[:], st.w2_h[f * 128:(f + 1) * 128, :])
            w2g.append(w2t)
        for tt in range(8):
            pg = ps_t("A", (128, 1024))
            for cc in range(2):
                sl = pg[:, cc * 512:(cc + 1) * 512]
                for ff in range(8):
                    nc.tensor.matmul(sl, ut_g[ff][:, tt * 128:(tt + 1) * 128],
                                     w2g[ff][:, cc * 512:(cc + 1) * 512],
                                     start=(ff == 0),
                                     stop=(False if g == 0 else ff == 7))
                if g == 0:
                    nc.tensor.matmul(sl, st.onesr[:, 0:128],
                                     st.b2_sb[:, cc * 512:(cc + 1) * 512],
                                     start=False, stop=True)
            if g == 0:
                nc.vector.tensor_tensor(y_acc[tt][:], pg[:], st.x2[tt][:],
                                        ALU.add)
            else:
                nc.vector.tensor_tensor(y_acc[tt][:], pg[:], y_acc[tt][:],
                                        ALU.add)
    for tt in range(8):
        nc.sync.dma_start(st.y_h[tt * 128:(tt + 1) * 128, :], y_acc[tt][:])
    utpp.__exit__(None, None, None)
    w2pp.__exit__(None, None, None)
    w1pp.__exit__(None, None, None)
    yacp.__exit__(None, None, None)
    st.h2p.__exit__(None, None, None)
    st.latebp.__exit__(None, None, None)
    st.x2p.__exit__(None, None, None)


def build_program():
    if "nc" in _CACHE:
        return _CACHE["nc"]
    nc = bacc.Bacc(None)
    st = S()

    st.x_h = nc.declare_dram_parameter("x", [T, C], f32, isOutput=False)
    st.xres_h = nc.declare_dram_parameter("xres", [TH, C], f32, isOutput=False)
    st.wq_h = nc.declare_dram_parameter("wq", [C, H], bf16, isOutput=False)
    st.wk_h = nc.declare_dram_parameter("wk", [C, H], bf16, isOutput=False)
    st.wv_h = nc.declare_dram_parameter("wv", [C, H], bf16, isOutput=False)
    bq_h = nc.declare_dram_parameter("bq", [1, H], bf16, isOutput=False)
    bk_h = nc.declare_dram_parameter("bk", [1, H], bf16, isOutput=False)
    bv_h = nc.declare_dram_parameter("bv", [1, H], bf16, isOutput=False)
    st.wo_h = nc.declare_dram_parameter("wo", [C, C], bf16, isOutput=False)
    st.w1_h = nc.declare_dram_parameter("w1", [C, F], bf16, isOutput=False)
    b1_h = nc.declare_dram_parameter("b1", [128, 32], f32, isOutput=False)
    st.w2_h = nc.declare_dram_parameter("w2", [F, C], bf16, isOutput=False)
    b2_h = nc.declare_dram_parameter("b2", [1, C], bf16, isOutput=False)
    ident_h = nc.declare_dram_parameter("ident", [128, 128], bf16, isOutput=False)
    tri_h = nc.declare_dram_parameter("tri", [128, 128], bf16, isOutput=False)
    onesr_h = nc.declare_dram_parameter("onesr", [1, 512], bf16, isOutput=False)
    sel_h = nc.declare_dram_parameter("sel", [128, 2], f32, isOutput=False)
    seln_h = nc.declare_dram_parameter("seln", [128, 2], f32, isOutput=False)
    st.y_h = nc.declare_dram_parameter("y", [TH, C], f32, isOutput=True)

    st.rs_in = [nc.dram_tensor(f"rs_in{hp}", [2, 128, TH], bf16)
                for hp in range(4)]
    st.rs_out = [nc.dram_tensor(f"rs_out{hp}", [128, TH], bf16)
                 for hp in range(4)]

    with tile.TileContext(nc) as tc, ExitStack() as stack:
        st.tc, st.stack = tc, stack
        cst = stack.enter_context(tc.tile_pool(name="const", bufs=1))
        ps = stack.enter_context(tc.tile_pool(name="ps", bufs=1, space="PSUM"))
        st.work = stack.enter_context(tc.tile_pool(name="work", bufs=4))
        st.ptp = stack.enter_context(tc.tile_pool(name="ptp", bufs=3))

        st.ident = cst.tile([128, 128], bf16, name="ident")
        st.tri = cst.tile([128, 128], bf16, name="tri")
        st.onesr = cst.tile([1, 512], bf16, name="onesr")
        st.bq_sb = cst.tile([1, H], bf16, name="bq_sb")
        st.bk_sb = cst.tile([1, H], bf16, name="bk_sb")
        st.bv_sb = cst.tile([1, H], bf16, name="bv_sb")
        st.b1_sb = cst.tile([128, 32], f32, name="b1_sb")
        st.sel_sb = cst.tile([128, 2], f32, name="sel_sb")
        st.seln_sb = cst.tile([128, 2], f32, name="seln_sb")
        for t_, h_ in [(st.ident, ident_h), (st.tri, tri_h), (st.onesr, onesr_h),
                       (st.bq_sb, bq_h), (st.bk_sb, bk_h),
                       (st.bv_sb, bv_h), (st.b1_sb, b1_h),
                       (st.sel_sb, sel_h), (st.seln_sb, seln_h)]:
            nc.sync.dma_start(t_[:], h_[:])
        st.b2_h = b2_h

        def ps_t(tag, shape=(128, 512), dt=f32):
            if tag == "A":
                assert shape[0] <= 128 and shape[1] <= 1024
                full = ps.tile([128, 1024], dt, tag="A", name="ps_A", bufs=2)
                return full[0:shape[0], 0:shape[1]]
            tag = "B" if tag == "pv" else tag
            return ps.tile(list(shape), dt, tag="B", name="ps_B", bufs=4)
        st.ps_t = ps_t

        _phase_hcol(nc, st)
        _phase_attention(nc, st)
        _phase_proj(nc, st)
        _phase_ln2(nc, st)
        _phase_ffn(nc, st)

    nc.compile()
    _CACHE["nc"] = nc
    return nc


def make_inputs(x, Wq, Wk, Wv, Wo, bo, W1, b1, W2, b2,
                ln1_g, ln1_b, ln2_g, ln2_b):
    """Build per-core input maps (host-side sharding + LN folding)."""
    x = np.asarray(x, np.float32)
    scale = float(C) ** -0.5

    wq_eff = ln1_g[:, None] * Wq
    wk_eff = ln1_g[:, None] * Wk * scale
    wv_eff = ln1_g[:, None] * Wv
    bq_full = ln1_b @ Wq
    bk_full = (ln1_b @ Wk) * scale
    bv_full = ln1_b @ Wv
    w1_eff = ln2_g[:, None] * W1
    b1_eff = b1 + ln2_b @ W1

    BF = ml_dtypes.bfloat16
    ident = np.eye(128, dtype=BF)
    tri = np.triu(np.ones((128, 128), BF))
    onesr = np.ones((1, 512), BF)

    in_maps = []
    for core in range(8):
        b, s = core // 2, core % 2
        cs = slice(s * H, (s + 1) * H)
        ts = slice(s * TH, (s + 1) * TH)
        own = np.arange(s * H, (s + 1) * H)
        other = np.arange((1 - s) * H, (2 - s) * H)
        perm = np.concatenate([own, other])
        in_maps.append({
            "x": np.ascontiguousarray(x[b]),
            "xres": np.ascontiguousarray(x[b, ts, :] + bo[None, :]),
            "wq": np.ascontiguousarray(wq_eff[:, cs].astype(BF)),
            "wk": np.ascontiguousarray(wk_eff[:, cs].astype(BF)),
            "wv": np.ascontiguousarray(wv_eff[:, cs].astype(BF)),
            "bq": np.ascontiguousarray(bq_full[cs].reshape(1, H).astype(BF)),
            "bk": np.ascontiguousarray(bk_full[cs].reshape(1, H).astype(BF)),
            "bv": np.ascontiguousarray(bv_full[cs].reshape(1, H).astype(BF)),
            "wo": np.ascontiguousarray(Wo[perm, :].astype(BF)),
            "w1": np.ascontiguousarray(w1_eff.astype(BF)),
            "b1": np.ascontiguousarray(b1_eff.reshape(32, 128).T),
            "w2": np.ascontiguousarray(W2.astype(BF)),
            "b2": np.ascontiguousarray(b2.reshape(1, C).astype(BF)),
            "ident": ident, "tri": tri, "onesr": onesr,
            "sel": np.tile(np.eye(2, dtype=np.float32)[s][None, :], (128, 1)),
            "seln": np.tile(np.eye(2, dtype=np.float32)[1 - s][None, :], (128, 1)),
        })
    return in_maps


def kernel(**inputs):
    nc = build_program()
    in_maps = make_inputs(**{k: np.asarray(v, np.float32) for k, v in inputs.items()})
    res = run_bass_kernel_spmd(nc, in_maps, list(range(8)))
    out = np.empty((B, T, C), np.float32)
    for core in range(8):
        b, s = core // 2, core % 2
        out[b, s * TH:(s + 1) * TH, :] = res.results[core]["y"]
    return out
